# revision 1
# baseline (speedup 1.0000x reference)
"""Trainium2 Bass kernel for nn_NeuralMemory (Titans-style neural memory).

Sharding: 8 cores <-> 8 (batch, head) pairs. Each core runs the full
per-(b,h) pipeline; the host sums the 4 head partials per batch.

Math restructuring (validated vs the jax reference in fp64 at ~8e-6):
  - rmsnorm gains folded into projection weights (host-side).
  - inner-loss grads derived manually at the shared initial fast weights;
    the 2/DH*lr factor is dropped for g1/g2 (Newton-Schulz is
    scale-invariant) and applied only to the gamma grad.
  - Newton-Schulz-5 runs directly in the sigma domain on t = -g/nrm
    (t <- a t + (b A + c A^2) t, A = t t^T): numerically stable in fp16,
    unlike a Gram-space (A-only) formulation whose squared conditioning
    amplifies input rounding noise through negative-eigenvalue divergence.
  - momentum/decay scans fused per chunk with retrieval (which uses the
    weights from the end of the previous chunk).

Layouts: feature-major [feature, token] activations. fp16 matmul operands
(fp32 PSUM accumulation) except the h_pre matmul which runs in fp32r.
Big token-major packs and the per-chunk normalized grads are staged via
DRAM to stay inside SBUF.
"""
import sys

sys.path.insert(0, "/opt/trn_rl_repo")

import numpy as np

import concourse.bass as bass
import concourse.bacc as bacc
import concourse.mybir as mybir
import concourse.tile as tile
from concourse.bass import ts

F32 = mybir.dt.float32
F32R = mybir.dt.float32r
F16 = mybir.dt.float16

DIM, HEADS, DH, CHUNK = 512, 4, 128, 64
HID = DH * 4
B, N = 2, 2048
NCH = N // CHUNK          # 32 chunks
NTT = N // 512            # 4 token tiles
NSA, NSB, NSC = 3.4445, -4.775, 2.0315
AX = mybir.AluOpType
AF = mybir.ActivationFunctionType
X_AXIS = mybir.AxisListType.X
NGRP = 8                  # chunks per NS group (16 NS instances)

DEBUG = False


def build(nc):
    d = {}
    d["seqT"] = nc.dram_tensor("seqT", [DIM, N], F32, kind="ExternalInput")
    for nm in ["wk", "wv", "wq"]:
        d[nm] = nc.dram_tensor(nm, [128, 512], F32, kind="ExternalInput")
    d["wsm"] = nc.dram_tensor("wsm", [128, 16], F32, kind="ExternalInput")
    d["biasB"] = nc.dram_tensor("biasB", [128, 4], F32, kind="ExternalInput")
    d["bias_md"] = nc.dram_tensor("bias_md", [2, 1], F32, kind="ExternalInput")
    d["w1"] = nc.dram_tensor("w1", [128, 512], F32, kind="ExternalInput")
    d["w2"] = nc.dram_tensor("w2", [128, 512], F32, kind="ExternalInput")
    d["w2T"] = nc.dram_tensor("w2T", [128, 512], F32, kind="ExternalInput")
    d["gamma"] = nc.dram_tensor("gamma", [128, 1], F32, kind="ExternalInput")
    d["wc"] = nc.dram_tensor("wc", [128, 512], F32, kind="ExternalInput")
    d["ones"] = nc.dram_tensor("ones", [128, 128], F32, kind="ExternalInput")
    d["aI4"] = nc.dram_tensor("aI4", [128, 512], F32, kind="ExternalInput")
    d["ident"] = nc.dram_tensor("ident", [128, 128], F32, kind="ExternalInput")
    d["partialT"] = nc.dram_tensor("partialT", [DIM, N], F32, kind="ExternalOutput")
    if DEBUG:
        for nm, shp, dt in [
            ("dbg_rs", [1, N], F32), ("dbg_kT", [128, 512], F32),
            ("dbg_hh", [128, 512], F32), ("dbg_dhh", [128, 512], F16),
            ("dbg_g1", [128, 512], F16), ("dbg_g2", [128, 512], F16),
            ("dbg_F1", [128, 128], F16), ("dbg_u1", [128, 512], F32),
            ("dbg_outT", [128, N], F16), ("dbg_lr", [128, 512], F32),
            ("dbg_mom", [128, NCH], F32), ("dbg_gG", [128, NCH], F32),
            ("dbg_x0", [128, 128], F16),
            ("dbg_g1r", [128, 512], F16), ("dbg_s1", [128, 512], F16),
            ("dbg_u1c0", [128, 512], F32), ("dbg_s2", [128, 512], F32),
        ]:
            d[nm] = nc.dram_tensor(nm, shp, dt, kind="ExternalOutput")

    with tile.TileContext(nc) as tc:
        _body(nc, tc, d)
    return nc


def _body(nc, tc, d):
    def dma(out, in_):
        nc.sync.dma_start(out=out, in_=in_)

    consts_cm = tc.tile_pool(name="consts", bufs=1)
    persist_cm = tc.tile_pool(name="persist", bufs=1)
    dram_cm = tc.tile_pool(name="dstage", bufs=1, space="DRAM")
    with consts_cm as consts, persist_cm as persist, dram_cm as dstage:
        # ---------------- constants ----------------
        rawpool_cm = tc.tile_pool(name="rawpool", bufs=1)
        rawpool = rawpool_cm.__enter__()
        raw = {}
        for nm, shp in [("wk", [128, 4, 128]), ("wv", [128, 4, 128]),
                        ("wq", [128, 4, 128]), ("wsm", [128, 4, 4]),
                        ("w2", [128, 4, 128]), ("w1", [128, 512]),
                        ("w2T", [128, 512]), ("wc", [128, 512]),
                        ("ones", [128, 128]), ("aI4", [128, 4, 128]),
                        ("ident", [128, 128])]:
            pool_for = consts if nm in ("w1", "w2") else rawpool
            t = pool_for.tile(shp, F32, name=f"raw_{nm}")
            src = d[nm].ap()
            if len(shp) == 3:
                dma(t.rearrange("p a b -> p (a b)"), src)
            else:
                dma(t, src)
            raw[nm] = t
        biasB = consts.tile([128, 4], F32)
        dma(biasB, d["biasB"].ap())
        bias_md = consts.tile([2, 1], F32)
        dma(bias_md, d["bias_md"].ap())
        gamma = consts.tile([128, 1], F32)
        dma(gamma, d["gamma"].ap())
        epsT = consts.tile([128, 1], F32)
        nc.vector.memset(epsT, 1e-6)

        wk_h = consts.tile([128, 4, 128], F16)
        wv_h = consts.tile([128, 4, 128], F16)
        wq_h = consts.tile([128, 4, 128], F16)
        wsm_h = consts.tile([128, 4, 4], F16)
        w1_r = consts.tile([128, 512], F32R)
        w2_h = consts.tile([128, 4, 128], F16)
        w2T_h = consts.tile([128, 512], F16)
        wc_h = consts.tile([128, 512], F16)
        ones_col_h = consts.tile([128, 1], F16)
        ones_row_h = consts.tile([1, 128], F16)
        aI4_h = consts.tile([128, 4, 128], F16)
        ident_h = consts.tile([128, 128], F16)
        for dst, src in [(wk_h, raw["wk"]), (wv_h, raw["wv"]), (wq_h, raw["wq"]),
                         (wsm_h, raw["wsm"]), (w1_r, raw["w1"]), (w2_h, raw["w2"]),
                         (w2T_h, raw["w2T"]), (wc_h, raw["wc"])]:
            nc.vector.tensor_copy(out=dst, in_=src)
        nc.vector.tensor_copy(out=ones_col_h, in_=raw["ones"][:, 0:1])
        nc.vector.tensor_copy(out=ones_row_h, in_=raw["ones"][0:1, :])
        nc.vector.tensor_copy(out=aI4_h, in_=raw["aI4"])
        nc.vector.tensor_copy(out=ident_h, in_=raw["ident"])
        rawpool_cm.__exit__(None, None, None)

        # -------- persistent tiles + DRAM staging --------
        qT_h = persist.tile([128, N], F16)
        gateB = persist.tile([128, N], F32)
        mdraw = persist.tile([2, NCH], F32)
        momB = persist.tile([128, NCH], F32)
        decm1B = persist.tile([128, NCH], F32)
        gG = persist.tile([128, NCH], F32)
        kc_st = dstage.tile([64, NCH, 128], F16)
        dhh_st = dstage.tile([64, NCH, 128], F16)
        dhpre_st = dstage.tile([64, NCH, 512], F16)
        hact_st = dstage.tile([64, NCH, 512], F16)
        s1_st = dstage.tile([NCH, 128, 512], F16)
        s2_st = dstage.tile([NCH, 128, 512], F16)

        # ================= PHASE A: store-side, streamed per token-tile ========
        with tc.tile_pool(name="phA", bufs=1) as pA, \
             tc.tile_pool(name="psA", bufs=1, space="PSUM") as psA:
            for tt in range(NTT):
                tsl = ts(tt, 512)
                seq_t = pA.tile([128, 4, 512], F32, tag="seq_t", bufs=2)
                dma(seq_t,
                    d["seqT"].ap().rearrange("(a p) t -> p a t", p=128)[:, :, tsl])
                # rmsnorm scale
                ss_ps = psA.tile([1, 512], F32, tag="mix", bufs=2)
                for j in range(4):
                    sqs = pA.tile([128, 512], F16, tag="sqs", bufs=2)
                    nc.scalar.activation(out=sqs, in_=seq_t[:, j, :], func=AF.Square)
                    nc.tensor.matmul(ss_ps, ones_col_h, sqs,
                                     start=(j == 0), stop=(j == 3))
                rowt = pA.tile([1, 512], F32, tag="rows", bufs=10)
                nc.scalar.activation(out=rowt, in_=ss_ps, func=AF.Sqrt,
                                     scale=1.0 / DIM, bias=epsT[0:1, :])
                rs_f = pA.tile([1, 512], F32, tag="rows", bufs=10)
                nc.vector.reciprocal(out=rs_f, in_=rowt)
                rs_h = pA.tile([1, 512], F16, tag="rows", bufs=10)
                nc.scalar.copy(out=rs_h, in_=rs_f)
                if DEBUG:
                    dma(d["dbg_rs"].ap()[:, tsl], rs_f)
                rsb_ps = psA.tile([128, 512], F32, tag="bc", bufs=2)
                nc.tensor.matmul(rsb_ps, ones_row_h, rs_h, start=True, stop=True)
                sT_t = pA.tile([128, 4, 512], F16, tag="sT_t", bufs=2)
                for j in range(4):
                    nc.vector.tensor_mul(out=sT_t[:, j, :], in0=seq_t[:, j, :],
                                         in1=rsb_ps)

                # projections
                k_ps = psA.tile([128, 512], F32, tag="proj", bufs=2)
                for j in range(4):
                    nc.tensor.matmul(k_ps, wk_h[:, j, :], sT_t[:, j, :],
                                     start=(j == 0), stop=(j == 3))
                kT_r = pA.tile([128, 512], F32R, tag="kT_r")
                nc.vector.tensor_copy(out=kT_r, in_=k_ps)
                kT_h = pA.tile([128, 512], F16, tag="kT_h")
                nc.scalar.copy(out=kT_h, in_=k_ps)
                if DEBUG and tt == 0:
                    dma(d["dbg_kT"].ap(), kT_r.bitcast(F32))
                v_ps = psA.tile([128, 512], F32, tag="proj", bufs=2)
                for j in range(4):
                    nc.tensor.matmul(v_ps, wv_h[:, j, :], sT_t[:, j, :],
                                     start=(j == 0), stop=(j == 3))
                kvT = pA.tile([128, 512], F32, tag="kvT")
                nc.vector.tensor_sub(out=kvT, in0=kT_r.bitcast(F32), in1=v_ps)
                q_ps = psA.tile([128, 512], F32, tag="proj", bufs=2)
                for j in range(4):
                    nc.tensor.matmul(q_ps, wq_h[:, j, :], sT_t[:, j, :],
                                     start=(j == 0), stop=(j == 3))
                nc.scalar.copy(out=qT_h[:, tsl], in_=q_ps)
                sm_ps = psA.tile([4, 512], F32, tag="mix", bufs=2)
                for j in range(4):
                    nc.tensor.matmul(sm_ps, wsm_h[:, j, :], sT_t[:, j, :],
                                     start=(j == 0), stop=(j == 3))
                # copy to sbuf, then extract rows at partition 0 via tiny DMAs
                smsb = pA.tile([4, 512], F32, tag="smsb", bufs=2)
                nc.vector.tensor_copy(out=smsb, in_=sm_ps)
                lr_row = pA.tile([1, 512], F32, tag="rows", bufs=10)
                gt_row = pA.tile([1, 512], F32, tag="rows", bufs=10)
                md_rows = pA.tile([2, 512], F32, tag="md_rows", bufs=2)
                dma(lr_row, smsb[0:1, :])
                dma(gt_row, smsb[3:4, :])
                dma(md_rows, smsb[1:3, :])
                nc.vector.tensor_reduce(
                    out=mdraw[:, tt * 8:(tt + 1) * 8],
                    in_=md_rows.rearrange("p (c k) -> p c k", k=CHUNK),
                    axis=X_AXIS, op=AX.add)
                lr_h = pA.tile([1, 512], F16, tag="rows", bufs=10)
                nc.scalar.copy(out=lr_h, in_=lr_row)
                gt_h = pA.tile([1, 512], F16, tag="rows", bufs=10)
                nc.scalar.copy(out=gt_h, in_=gt_row)
                lg_ps = psA.tile([128, 512], F32, tag="bc", bufs=2)
                nc.tensor.matmul(lg_ps, ones_row_h, lr_h, start=True, stop=True)
                lrB = pA.tile([128, 512], F32, tag="lrB")
                nc.scalar.activation(out=lrB, in_=lg_ps, func=AF.Sigmoid,
                                     bias=biasB[:, 0:1])
                gt_ps = psA.tile([128, 512], F32, tag="bc", bufs=2)
                nc.tensor.matmul(gt_ps, ones_row_h, gt_h, start=True, stop=True)
                nc.scalar.activation(out=gateB[:, tsl], in_=gt_ps, func=AF.Sigmoid)
                if DEBUG and tt == 0:
                    dma(d["dbg_lr"].ap(), lrB)

                # forward MLP (h_pre in fp32r, rest fp16)
                hact_h = pA.tile([128, 4, 512], F16, tag="hact_h")
                dgel = pA.tile([128, 4, 512], F32, tag="dgel")
                for j in range(4):
                    hp_ps = psA.tile([128, 512], F32, tag="proj", bufs=2)
                    nc.tensor.matmul(hp_ps, w1_r[:, ts(j, 128)], kT_r,
                                     start=True, stop=True)
                    nc.scalar.activation(out=hact_h[:, j, :], in_=hp_ps,
                                         func=AF.Gelu)
                    nc.scalar.activation(out=dgel[:, j, :], in_=hp_ps,
                                         func=AF.Derivative_Gelu)
                hh_ps = psA.tile([128, 512], F32, tag="proj", bufs=2)
                for j in range(4):
                    nc.tensor.matmul(hh_ps, w2_h[:, j, :], hact_h[:, j, :],
                                     start=(j == 0), stop=(j == 3))
                hhsb = pA.tile([128, 512], F32, tag="hhsb")
                nc.vector.tensor_copy(out=hhsb, in_=hh_ps)
                if DEBUG and tt == 0:
                    dma(d["dbg_hh"].ap(), hhsb)
                sq2 = pA.tile([128, 512], F16, tag="sq2", bufs=2)
                nc.scalar.activation(out=sq2, in_=hh_ps, func=AF.Square)
                ms_ps = psA.tile([1, 512], F32, tag="mix", bufs=2)
                nc.tensor.matmul(ms_ps, ones_col_h, sq2, start=True, stop=True)
                rowt2 = pA.tile([1, 512], F32, tag="rows", bufs=10)
                nc.scalar.activation(out=rowt2, in_=ms_ps, func=AF.Sqrt,
                                     scale=1.0 / DH, bias=epsT[0:1, :])
                srs_f = pA.tile([1, 512], F32, tag="rows", bufs=10)
                nc.vector.reciprocal(out=srs_f, in_=rowt2)
                srs_h = pA.tile([1, 512], F16, tag="rows", bufs=10)
                nc.scalar.copy(out=srs_h, in_=srs_f)
                srsb_ps = psA.tile([128, 512], F32, tag="bc", bufs=2)
                nc.tensor.matmul(srsb_ps, ones_row_h, srs_h, start=True, stop=True)
                ysb = pA.tile([128, 512], F32, tag="ysb")
                nc.vector.tensor_mul(out=ysb, in0=hhsb, in1=srsb_ps)
                dp = pA.tile([128, 512], F32, tag="dp")
                nc.vector.scalar_tensor_tensor(out=dp, in0=ysb, scalar=gamma,
                                               in1=kvT, op0=AX.mult, op1=AX.add)
                nc.vector.tensor_mul(out=dp, in0=dp, in1=lrB)
                gp = pA.tile([128, 512], F32, tag="gp", bufs=2)
                nc.vector.tensor_mul(out=gp, in0=dp, in1=ysb)
                nc.vector.tensor_reduce(out=gG[:, tt * 8:(tt + 1) * 8],
                                        in_=gp.rearrange("p (c k) -> p c k", k=CHUNK),
                                        axis=X_AXIS, op=AX.add)
                dY = pA.tile([128, 512], F32, tag="dY")
                nc.vector.tensor_scalar_mul(out=dY, in0=dp, scalar1=gamma)
                dprod = pA.tile([128, 512], F16, tag="dprod", bufs=2)
                nc.vector.tensor_mul(out=dprod, in0=dY, in1=hhsb)
                dot_ps = psA.tile([1, 512], F32, tag="mix", bufs=2)
                nc.tensor.matmul(dot_ps, ones_col_h, dprod, start=True, stop=True)
                s3 = pA.tile([1, 512], F32, tag="rows", bufs=10)
                nc.vector.tensor_mul(out=s3, in0=srs_f, in1=srs_f)
                nc.vector.tensor_mul(out=s3, in0=s3, in1=srs_f)
                c_f = pA.tile([1, 512], F32, tag="rows", bufs=10)
                nc.vector.tensor_mul(out=c_f, in0=s3, in1=dot_ps)
                c_h = pA.tile([1, 512], F16, tag="rows", bufs=10)
                nc.scalar.activation(out=c_h, in_=c_f, func=AF.Copy, scale=1.0 / DH)
                cb_ps = psA.tile([128, 512], F32, tag="bc", bufs=2)
                nc.tensor.matmul(cb_ps, ones_row_h, c_h, start=True, stop=True)
                m1t = pA.tile([128, 512], F32, tag="m1t", bufs=2)
                nc.vector.tensor_mul(out=m1t, in0=dY, in1=srsb_ps)
                m2t = pA.tile([128, 512], F32, tag="m2t", bufs=2)
                nc.vector.tensor_mul(out=m2t, in0=hhsb, in1=cb_ps)
                dhh_h = pA.tile([128, 512], F16, tag="dhh_h")
                nc.vector.tensor_sub(out=dhh_h, in0=m1t, in1=m2t)
                if DEBUG and tt == 0:
                    dma(d["dbg_dhh"].ap(), dhh_h)

                # backward to dhpre (fp16)
                dhpre_h = pA.tile([128, 4, 512], F16, tag="dhpre_h")
                for j in range(4):
                    da_ps = psA.tile([128, 512], F32, tag="proj", bufs=2)
                    nc.tensor.matmul(da_ps, w2T_h[:, ts(j, 128)], dhh_h,
                                     start=True, stop=True)
                    nc.vector.tensor_mul(out=dhpre_h[:, j, :], in0=da_ps,
                                         in1=dgel[:, j, :])

                # token-major transposes (fp16) -> staging -> chunk-major DRAM
                st_kc = pA.tile([128, 4, 128], F16, tag="st_kc", bufs=1)
                st_dh = pA.tile([128, 4, 128], F16, tag="st_dh", bufs=1)
                st_dp = pA.tile([128, 4, 512], F16, tag="st_dp", bufs=1)
                st_ha = pA.tile([128, 4, 512], F16, tag="st_ha", bufs=1)
                for blk in range(4):
                    bsl = ts(blk, 128)
                    tp_ps = psA.tile([128, 4, 128], F16, tag="tp", bufs=2)
                    nc.tensor.transpose(tp_ps[:, 0, :], kT_h[:, bsl], ident_h)
                    nc.tensor.transpose(tp_ps[:, 1, :], dhh_h[:, bsl], ident_h)
                    nc.vector.tensor_copy(out=st_kc[:, blk, :], in_=tp_ps[:, 0, :])
                    nc.vector.tensor_copy(out=st_dh[:, blk, :], in_=tp_ps[:, 1, :])
                    for j in range(4):
                        t2_ps = psA.tile([128, 4, 128], F16, tag="tp", bufs=2)
                        nc.tensor.transpose(t2_ps[:, 0, :], dhpre_h[:, j, bsl],
                                            ident_h)
                        nc.tensor.transpose(t2_ps[:, 1, :], hact_h[:, j, bsl],
                                            ident_h)
                        nc.vector.tensor_copy(out=st_dp[:, blk, ts(j, 128)],
                                              in_=t2_ps[:, 0, :])
                        nc.vector.tensor_copy(out=st_ha[:, blk, ts(j, 128)],
                                              in_=t2_ps[:, 1, :])
                for cm, stg in [(kc_st, st_kc), (dhh_st, st_dh),
                                (dhpre_st, st_dp), (hact_st, st_ha)]:
                    v = cm.rearrange("p (a two) x -> p a two x", two=2)
                    dma(v[:, 4 * tt:4 * tt + 4, 0, :], stg[0:64, :, :])
                    dma(v[:, 4 * tt:4 * tt + 4, 1, :], stg[64:128, :, :])

            # finish mom/dec (all chunks)
            mds = pA.tile([2, NCH], F32, tag="mds")
            nc.scalar.activation(out=mds, in_=mdraw, func=AF.Sigmoid,
                                 scale=1.0 / CHUNK, bias=bias_md)
            mrow_f = pA.tile([1, NCH], F32, tag="mrow_f")
            drow_f = pA.tile([1, NCH], F32, tag="drow_f")
            dma(mrow_f, mds[0:1, :])
            dma(drow_f, mds[1:2, :])
            mrow = pA.tile([1, NCH], F16, tag="mrow")
            drow = pA.tile([1, NCH], F16, tag="drow")
            nc.scalar.copy(out=mrow, in_=mrow_f)
            nc.scalar.copy(out=drow, in_=drow_f)
            mb_ps = psA.tile([128, 512], F32, tag="bc", bufs=2)
            nc.tensor.matmul(mb_ps[:, 0:NCH], ones_row_h, mrow, start=True, stop=True)
            nc.tensor.matmul(mb_ps[:, 64:64 + NCH], ones_row_h, drow,
                             start=True, stop=True)
            nc.vector.tensor_copy(out=momB, in_=mb_ps[:, 0:NCH])
            nc.scalar.activation(out=decm1B, in_=mb_ps[:, 64:64 + NCH],
                                 func=AF.Identity, scale=-1.0, bias=1.0)
            nc.vector.tensor_scalar_mul(out=gG, in0=gG, scalar1=-2.0 / DH)
            if DEBUG:
                dma(d["dbg_mom"].ap(), momB)
                dma(d["dbg_gG"].ap(), gG)

        # ================= PHASE B: grads + Gram-space NS5 =====================
        with tc.tile_pool(name="phB", bufs=1) as pB, \
             tc.tile_pool(name="psB", bufs=1, space="PSUM") as psB:
            for g in range(NCH // NGRP):
                chs = list(range(g * NGRP, (g + 1) * NGRP))
                n_inst = 2 * NGRP
                gsl = ts(g, NGRP)
                kc_g = pB.tile([64, NGRP, 128], F16, tag="kc_g", bufs=2)
                dma(kc_g, kc_st[:, gsl, :])
                dhh_g = pB.tile([64, NGRP, 128], F16, tag="dhh_g", bufs=2)
                dma(dhh_g, dhh_st[:, gsl, :])
                dhpre_g = pB.tile([64, NGRP, 512], F16, tag="dhpre_g", bufs=2)
                dma(dhpre_g, dhpre_st[:, gsl, :])
                hact_g = pB.tile([64, NGRP, 512], F16, tag="hact_g", bufs=2)
                dma(hact_g, hact_st[:, gsl, :])
                R = pB.tile([128, n_inst], F32, tag="R", bufs=2)
                gsb = pB.tile([128, n_inst, 512], F16, tag="gsb", bufs=1)
                for ii, c in enumerate(chs):
                    kc_l = kc_g[:, ii, :]
                    dhp_l = dhpre_g[:, ii, :]
                    dhh_l = dhh_g[:, ii, :]
                    ha_l = hact_g[:, ii, :]
                    g_ps = psB.tile([128, 512], F32, tag="g", bufs=2)
                    nc.tensor.matmul(g_ps, kc_l, dhp_l, start=True, stop=True)
                    nc.vector.tensor_copy(out=gsb[:, 2 * ii, :], in_=g_ps)
                    scr = pB.tile([128, 512], F16, tag="scr", bufs=2)
                    nc.vector.scalar_tensor_tensor(
                        out=scr, in0=gsb[:, 2 * ii, :], scalar=1.0,
                        in1=gsb[:, 2 * ii, :], op0=AX.mult, op1=AX.mult,
                        accum_out=R[:, 2 * ii:2 * ii + 1])
                    g2_ps = psB.tile([128, 512], F32, tag="g", bufs=2)
                    nc.tensor.matmul(g2_ps, dhh_l, ha_l, start=True, stop=True)
                    nc.vector.tensor_copy(out=gsb[:, 2 * ii + 1, :], in_=g2_ps)
                    scr2 = pB.tile([128, 512], F16, tag="scr", bufs=2)
                    nc.vector.scalar_tensor_tensor(
                        out=scr2, in0=gsb[:, 2 * ii + 1, :], scalar=1.0,
                        in1=gsb[:, 2 * ii + 1, :], op0=AX.mult, op1=AX.mult,
                        accum_out=R[:, 2 * ii + 1:2 * ii + 2])
                    if DEBUG and c == 0:
                        dma(d["dbg_g1"].ap(), gsb[:, 0, :])
                        dma(d["dbg_g2"].ap(), gsb[:, 1, :])
                # norms
                Rh = pB.tile([128, n_inst], F16, tag="Rh", bufs=2)
                nc.vector.tensor_copy(out=Rh, in_=R)
                nrm_ps = psB.tile([1, n_inst], F32, tag="nrm", bufs=2)
                for i2 in range(n_inst):
                    nc.tensor.matmul(nrm_ps[:, i2:i2 + 1], ones_col_h,
                                     Rh[:, i2:i2 + 1], start=True, stop=True)
                inv2 = pB.tile([1, n_inst], F32, tag="inv2", bufs=2)
                nc.vector.reciprocal(out=inv2, in_=nrm_ps)
                ninv = pB.tile([1, n_inst], F32, tag="ninv", bufs=2)
                nc.scalar.activation(out=ninv, in_=inv2, func=AF.Sqrt)
                nc.scalar.activation(out=ninv, in_=ninv, func=AF.Copy, scale=-1.0)
                nb = pB.tile([128, n_inst], F32, tag="nb", bufs=2)
                nc.gpsimd.partition_broadcast(nb, ninv)

                # direct sigma-domain NS5 on t = -g/nrm (fp16, stable)
                for i2 in range(n_inst):
                    c = chs[i2 // 2]
                    tP = pB.tile([128, 512], F16, tag="tP", bufs=2)
                    nc.vector.tensor_scalar_mul(out=tP, in0=gsb[:, i2, :],
                                                scalar1=nb[:, i2:i2 + 1])
                    tT = pB.tile([128, 4, 128], F16, tag="tT", bufs=2)
                    for j in range(4):
                        tt_ps = psB.tile([128, 128], F16, tag="ttp", bufs=2)
                        nc.tensor.transpose(tt_ps, tP[:, ts(j, 128)], ident_h)
                        nc.vector.tensor_copy(out=tT[:, j, :], in_=tt_ps)
                    for k in range(5):
                        A_ps = psB.tile([128, 128], F32, tag="x2", bufs=2)
                        for j in range(4):
                            nc.tensor.matmul(A_ps, tT[:, j, :], tT[:, j, :],
                                             start=(j == 0), stop=(j == 3))
                        Ab = pB.tile([128, 128], F16, tag="Ab", bufs=2)
                        nc.vector.tensor_scalar_mul(out=Ab, in0=A_ps, scalar1=NSB)
                        Au = pB.tile([128, 128], F16, tag="Au", bufs=2)
                        nc.vector.tensor_copy(out=Au, in_=A_ps)
                        A2_ps = psB.tile([128, 128], F32, tag="x2", bufs=2)
                        nc.tensor.matmul(A2_ps, Ab, Au, start=True, stop=True)
                        Bm = pB.tile([128, 128], F16, tag="Bm", bufs=2)
                        # Bm = (b*A2)*(c/b) + b*A = c*A2 + b*A
                        nc.vector.scalar_tensor_tensor(
                            out=Bm, in0=A2_ps, scalar=NSC / NSB, in1=Ab,
                            op0=AX.mult, op1=AX.add)
                        Bt_ps = psB.tile([128, 512], F32, tag="g", bufs=2)
                        nc.tensor.matmul(Bt_ps, Bm, tP, start=True, stop=True)
                        tPn = pB.tile([128, 512], F16, tag="tP", bufs=2)
                        nc.vector.scalar_tensor_tensor(
                            out=tPn, in0=tP, scalar=NSA, in1=Bt_ps,
                            op0=AX.mult, op1=AX.add)
                        tP = tPn
                        if k < 4:
                            tT = pB.tile([128, 4, 128], F16, tag="tT", bufs=2)
                            for j in range(4):
                                tt_ps = psB.tile([128, 128], F16, tag="ttp", bufs=2)
                                nc.tensor.transpose(tt_ps, tP[:, ts(j, 128)],
                                                    ident_h)
                                nc.vector.tensor_copy(out=tT[:, j, :], in_=tt_ps)
                    if i2 % 2 == 0:
                        dma(s1_st[c], tP)
                        if DEBUG and c == 0:
                            dma(d["dbg_F1"].ap(), tP[:, 0:128])
                    else:
                        # matrix 2: store native (hid, dh) layout via transpose
                        s2n = pB.tile([128, 4, 128], F16, tag="s2n", bufs=2)
                        for j in range(4):
                            tt_ps = psB.tile([128, 128], F16, tag="ttp", bufs=2)
                            nc.tensor.transpose(tt_ps, tP[:, ts(j, 128)], ident_h)
                            nc.vector.tensor_copy(out=s2n[:, j, :], in_=tt_ps)
                        dma(s2_st[c], s2n.rearrange("p a b -> p (a b)"))

        # ================= PHASE C: scans + retrieval + output ================
        with tc.tile_pool(name="phC", bufs=1) as pC, \
             tc.tile_pool(name="psC", bufs=1, space="PSUM") as psC:
            u1 = pC.tile([128, 512], F32, tag="u1")
            u2 = pC.tile([128, 4, 128], F32, tag="u2")
            m1s = pC.tile([128, 512], F32, tag="m1s")
            m2s = pC.tile([128, 4, 128], F32, tag="m2s")
            u1h = pC.tile([128, 512], F16, tag="u1h")
            u2h = pC.tile([128, 4, 128], F16, tag="u2h")
            ugv = pC.tile([128, 1], F32, tag="ugv")
            mgv = pC.tile([128, 1], F32, tag="mgv")
            outT = pC.tile([128, N], F16, tag="outT")
            nc.vector.tensor_copy(out=u1, in_=raw["w1"])
            nc.vector.tensor_copy(out=u2, in_=raw["w2"])
            nc.vector.tensor_copy(out=u1h, in_=raw["w1"])
            nc.vector.tensor_copy(out=u2h, in_=raw["w2"])
            nc.vector.tensor_copy(out=ugv, in_=gamma)
            nc.vector.memset(m1s, 0.0)
            nc.vector.memset(m2s, 0.0)
            nc.vector.memset(mgv, 0.0)

            for c in range(NCH):
                sl = ts(c, CHUNK)
                s1c = pC.tile([128, 512], F16, tag="s1c", bufs=4)
                dma(s1c, s1_st[c])
                s2c = pC.tile([128, 4, 128], F16, tag="s2c", bufs=4)
                dma(s2c.rearrange("p a b -> p (a b)"), s2_st[c])

                # retrieval with pre-update state
                hp_ps = psC.tile([128, 4, CHUNK], F32, tag="hp", bufs=2)
                for j in range(4):
                    nc.tensor.matmul(hp_ps[:, j, :], u1h[:, ts(j, 128)],
                                     qT_h[:, sl], start=True, stop=True)
                ha_c = pC.tile([128, 4, CHUNK], F16, tag="ha_c", bufs=2)
                nc.scalar.activation(out=ha_c, in_=hp_ps, func=AF.Gelu)
                hh_ps = psC.tile([128, CHUNK], F32, tag="csm", bufs=3)
                for j in range(4):
                    nc.tensor.matmul(hh_ps, u2h[:, j, :], ha_c[:, j, :],
                                     start=(j == 0), stop=(j == 3))
                sqc = pC.tile([128, CHUNK], F16, tag="sqc", bufs=2)
                nc.scalar.activation(out=sqc, in_=hh_ps, func=AF.Square)
                ms_ps = psC.tile([1, CHUNK], F32, tag="csm", bufs=3)
                nc.tensor.matmul(ms_ps, ones_col_h, sqc, start=True, stop=True)
                rr = pC.tile([1, CHUNK], F32, tag="rr", bufs=2)
                nc.scalar.activation(out=rr, in_=ms_ps, func=AF.Sqrt,
                                     scale=1.0 / DH, bias=epsT[0:1, :])
                rr2 = pC.tile([1, CHUNK], F32, tag="rr2", bufs=2)
                nc.vector.reciprocal(out=rr2, in_=rr)
                rrh = pC.tile([1, CHUNK], F16, tag="rrh", bufs=2)
                nc.scalar.copy(out=rrh, in_=rr2)
                sb_ps = psC.tile([128, CHUNK], F32, tag="csm", bufs=3)
                nc.tensor.matmul(sb_ps, ones_row_h, rrh, start=True, stop=True)
                hhc = pC.tile([128, CHUNK], F32, tag="hhc", bufs=2)
                nc.scalar.copy(out=hhc, in_=hh_ps)
                yc = pC.tile([128, CHUNK], F32, tag="yc", bufs=2)
                nc.vector.tensor_mul(out=yc, in0=hhc, in1=sb_ps)
                prc = pC.tile([128, CHUNK], F32, tag="prc", bufs=2)
                nc.vector.scalar_tensor_tensor(out=prc, in0=yc, scalar=ugv,
                                               in1=qT_h[:, sl],
                                               op0=AX.mult, op1=AX.add)
                nc.vector.tensor_mul(out=outT[:, sl], in0=prc, in1=gateB[:, sl])

                # scans (s already = NS output)
                if DEBUG and c == 0:
                    dma(d["dbg_s1"].ap(), s1c)
                nc.vector.scalar_tensor_tensor(out=m1s, in0=m1s,
                                               scalar=momB[:, c:c + 1], in1=s1c,
                                               op0=AX.mult, op1=AX.add)
                nc.vector.scalar_tensor_tensor(out=u1, in0=u1,
                                               scalar=decm1B[:, c:c + 1], in1=m1s,
                                               op0=AX.mult, op1=AX.add)
                nc.scalar.copy(out=u1h, in_=u1)
                if DEBUG and c == 0:
                    dma(d["dbg_u1c0"].ap(), u1)
                nc.vector.scalar_tensor_tensor(out=m2s, in0=m2s,
                                               scalar=momB[:, c:c + 1], in1=s2c,
                                               op0=AX.mult, op1=AX.add)
                nc.vector.scalar_tensor_tensor(out=u2, in0=u2,
                                               scalar=decm1B[:, c:c + 1], in1=m2s,
                                               op0=AX.mult, op1=AX.add)
                nc.scalar.copy(out=u2h, in_=u2)
                nc.vector.scalar_tensor_tensor(out=mgv, in0=mgv,
                                               scalar=momB[:, c:c + 1],
                                               in1=gG[:, c:c + 1],
                                               op0=AX.mult, op1=AX.add)
                nc.vector.scalar_tensor_tensor(out=ugv, in0=ugv,
                                               scalar=decm1B[:, c:c + 1], in1=mgv,
                                               op0=AX.mult, op1=AX.add)
            if DEBUG:
                dma(d["dbg_u1"].ap(), u1)
                dma(d["dbg_outT"].ap(), outT)

            # final projection
            for i in range(4):
                for tt in range(NTT):
                    o_ps = psC.tile([128, 512], F32, tag="sps", bufs=3)
                    nc.tensor.matmul(o_ps, wc_h[:, ts(i, 128)], outT[:, ts(tt, 512)],
                                     start=True, stop=True)
                    osb = pC.tile([128, 512], F32, tag="osb", bufs=3)
                    nc.scalar.copy(out=osb, in_=o_ps)
                    dma(d["partialT"].ap()[ts(i, 128), ts(tt, 512)], osb)


# ------------------- host side -------------------

def _prep_core_inputs(inputs, b, h):
    f = np.float32
    sg = np.asarray(inputs["store_g"], f)[:, None]
    rg = np.asarray(inputs["retrieve_g"], f)[:, None]
    hs = slice(h * DH, (h + 1) * DH)

    def tile128(w):  # (512, X) -> rows grouped as (128, 4, X) -> (128, 4*X)
        w = np.asarray(w, f)
        return np.ascontiguousarray(
            w.reshape(4, 128, -1).transpose(1, 0, 2).reshape(128, -1))

    wk = tile128(sg * np.asarray(inputs["Wk"], f)[:, hs])
    wv = tile128(sg * np.asarray(inputs["Wv"], f)[:, hs])
    wq = tile128(rg * np.asarray(inputs["Wq"], f)[:, hs])
    wsm = tile128(np.stack([
        sg[:, 0] * np.asarray(inputs["W_lr"], f)[:, h],
        sg[:, 0] * np.asarray(inputs["Wm"], f)[:, h],
        sg[:, 0] * np.asarray(inputs["Wd"], f)[:, h],
        rg[:, 0] * np.asarray(inputs["Wgate"], f)[:, h]], axis=1))
    biasB = np.broadcast_to(
        np.array([inputs["b_lr"][h], 0.0, 0.0, 0.0], f), (128, 4)).copy()
    bias_md = np.array([[inputs["bm"][h]], [inputs["bd"][h]]], f)
    w1 = np.asarray(inputs["mw1"], f)[h]
    w2 = tile128(np.asarray(inputs["mw2"], f)[h])
    w2T = np.ascontiguousarray(np.asarray(inputs["mw2"], f)[h].T)
    gamma = np.asarray(inputs["mgamma"], f)[h].reshape(128, 1)
    wc = np.ascontiguousarray(np.asarray(inputs["Wc"], f)[hs, :])
    seqT = np.ascontiguousarray(np.asarray(inputs["seq"], f)[b].T)
    ones = np.ones((128, 128), f)
    aI4 = np.tile((NSA * np.eye(128)).astype(f), (1, 4))
    ident = np.eye(128, dtype=f)
    return {
        "seqT": seqT, "wk": wk, "wv": wv, "wq": wq, "wsm": wsm,
        "biasB": biasB, "bias_md": bias_md, "w1": w1, "w2": w2, "w2T": w2T,
        "gamma": gamma, "wc": wc, "ones": ones, "aI4": aI4, "ident": ident,
    }


_CACHE = {}


def _get_module():
    if "nc" not in _CACHE:
        nc = bacc.Bacc("TRN2", target_bir_lowering=False, debug=False)
        build(nc)
        nc.compile()
        _CACHE["nc"] = nc
    return _CACHE["nc"]


def kernel(**inputs):
    from concourse.bass_utils import run_bass_kernel_spmd
    nc = _get_module()
    in_maps = [_prep_core_inputs(inputs, core // HEADS, core % HEADS)
               for core in range(8)]
    res = run_bass_kernel_spmd(nc, in_maps, core_ids=list(range(8)))
    _CACHE["last_res"] = res
    out = np.zeros((B, N, DIM), np.float32)
    for core in range(8):
        b = core // HEADS
        out[b] += res.results[core]["partialT"].T
    return out


if __name__ == "__main__":
    dd = np.load("/root/problem/ref_inputs.npz")
    inputs = {k: dd[k] for k in dd.files}
    out = kernel(**inputs)
    exp = np.load("/root/problem/ref_expected.npy")
    err = np.abs(out - exp).max() / np.abs(exp).max()
    rel = np.linalg.norm(out - exp) / np.linalg.norm(exp)
    print(f"absmax-rel: {err:.3e}  l2-rel: {rel:.3e}")



# revision 5
# speedup vs baseline: 3.5245x; 3.5245x over previous
"""Trainium2 Bass kernel for nn_NeuralMemory (Titans-style neural memory).

Sharding: 8 cores <-> 8 (batch, head) pairs. Each core runs the full
per-(b,h) pipeline; the host applies the final Wc projection and sums
the 4 head partials per batch (268 MFLOP of BLAS, ~ms).

This revision optimizes END-TO-END dispatch cost, which dominates the
measured time on the axon/PJRT path (per-call jit+compile-cache+load
scales with program size; transfers scale with I/O bytes):
  - I/O contract shrunk ~4x: fp16 seq, packed fp16/fp32 weight tensors,
    fp16 [NCH,128,64] gated output (pre-Wc); dead inputs dropped.
  - Instruction count ~25x smaller: the 32-chunk pipeline (inner grads,
    NS5, momentum/decay scans, retrieval) runs in ONE hardware For_i
    loop instead of being fully unrolled.
  - Newton-Schulz-5 runs in fp32 (f32r matmuls) — device time is cheap
    here, and it cuts rel-err ~15x vs the old fp16 NS (1e-3 vs 1.6e-2).
    Transpose-free NS: maintain t [dh,hid] and tT blocks; with
    Bm' = aI + bA + cA^2 (A = t t^T, symmetric), t_new = Bm' t and
    tT_new_j = t_j^T Bm' are plain matmuls off the OLD t — no PE
    transposes inside the iteration.

Math restructuring (validated vs the jax reference in numpy at ~1.4e-5
fp32 / ~1.0e-3 with fp16 phase A):
  - rmsnorm gains folded into projection weights (host-side).
  - inner-loss grads derived manually at the shared initial fast
    weights; the 2/DH*lr factor is dropped for g1/g2 (NS is
    scale-invariant) and applied only to the gamma grad.
  - momentum/decay scans fused per chunk with retrieval (which uses the
    weights from the end of the previous chunk).
"""
import sys

sys.path.insert(0, "/opt/trn_rl_repo")

import numpy as np

import concourse.bass as bass
import concourse.bacc as bacc
import concourse.mybir as mybir
import concourse.tile as tile
from concourse.bass import ts, ds

F32 = mybir.dt.float32
F32R = mybir.dt.float32r
F16 = mybir.dt.float16

DIM, HEADS, DH, CHUNK = 512, 4, 128, 64
HID = DH * 4
B, N = 2, 2048
NCH = N // CHUNK          # 32 chunks
NTT = N // 512            # 4 token tiles
NSA, NSB, NSC = 3.4445, -4.775, 2.0315
AX = mybir.AluOpType
AF = mybir.ActivationFunctionType
X_AXIS = mybir.AxisListType.X

# packed fp16 const columns: wk | wv | wq | wsm | ident
C16_WK, C16_WV, C16_WQ, C16_WSM, C16_ID = 0, 512, 1024, 1536, 1552
K16 = 1552 + 128
# packed fp32 const columns: w1 | w2 | gamma | biasB | md-bias col
C32_W1, C32_W2, C32_G, C32_BB, C32_MD = 0, 512, 1024, 1025, 1029
K32 = 1030


def build(nc):
    d = {}
    d["seqT"] = nc.dram_tensor("seqT", [DIM, N], F16, kind="ExternalInput")
    d["cw16"] = nc.dram_tensor("cw16", [128, K16], F16, kind="ExternalInput")
    d["cw32"] = nc.dram_tensor("cw32", [128, K32], F32, kind="ExternalInput")
    d["out"] = nc.dram_tensor("out", [NCH, 128, CHUNK], F16, kind="ExternalOutput")
    with tile.TileContext(nc) as tc:
        _body(nc, tc, d)
    return nc


def _body(nc, tc, d):
    def dma(out, in_):
        nc.sync.dma_start(out=out, in_=in_)

    consts_cm = tc.tile_pool(name="consts", bufs=1)
    persist_cm = tc.tile_pool(name="persist", bufs=1)
    dram_cm = tc.tile_pool(name="dstage", bufs=1, space="DRAM")
    with consts_cm as consts, persist_cm as persist, dram_cm as dstage:
        # ---------------- constants ----------------
        cw16 = consts.tile([128, K16], F16)
        dma(cw16, d["cw16"].ap())
        cw32 = consts.tile([128, K32], F32)
        dma(cw32, d["cw32"].ap())
        wk_h = cw16[:, C16_WK:C16_WK + 512]
        wv_h = cw16[:, C16_WV:C16_WV + 512]
        wq_h = cw16[:, C16_WQ:C16_WQ + 512]
        wsm_h = cw16[:, C16_WSM:C16_WSM + 16]
        ident_h = cw16[:, C16_ID:C16_ID + 128]
        w2_f = cw32[:, C32_W2:C32_W2 + 512]
        gamma = cw32[:, C32_G:C32_G + 1]
        biasB = cw32[:, C32_BB:C32_BB + 4]
        bias_md = cw32[0:2, C32_MD:C32_MD + 1]

        epsT = consts.tile([128, 1], F32)
        nc.vector.memset(epsT, 1e-6)
        ones_col_h = consts.tile([128, 1], F16)
        nc.vector.memset(ones_col_h, 1.0)
        ones_row_h = consts.tile([1, 128], F16)
        nc.vector.memset(ones_row_h, 1.0)
        identr = consts.tile([128, 128], F32R)
        nc.vector.tensor_copy(out=identr, in_=ident_h)
        aIc = consts.tile([128, 128], F32R)
        nc.scalar.activation(out=aIc, in_=identr.bitcast(F32), func=AF.Copy,
                             scale=NSA * NSB / NSC)
        w2_h = consts.tile([128, 512], F16)
        nc.vector.tensor_copy(out=w2_h, in_=w2_f)
        w1_r = consts.tile([128, 512], F32R)
        nc.vector.tensor_copy(out=w1_r, in_=cw32[:, C32_W1:C32_W1 + 512])

        # -------- persistent state --------
        u1 = persist.tile([128, 512], F32)
        u2 = persist.tile([128, 512], F32)
        u1h = persist.tile([128, 512], F16)
        u2h = persist.tile([128, 512], F16)
        m1s = persist.tile([128, 512], F32)
        m2s = persist.tile([128, 512], F32)
        ugv = persist.tile([128, 1], F32)
        mgv = persist.tile([128, 1], F32)
        mdraw = persist.tile([2, NCH], F32)
        gG = persist.tile([128, NCH], F32)
        w2T_h = persist.tile([128, 512], F16)
        nc.vector.tensor_copy(out=u1, in_=cw32[:, C32_W1:C32_W1 + 512])
        nc.vector.tensor_copy(out=u2, in_=w2_f)
        nc.vector.tensor_copy(out=u1h, in_=cw32[:, C32_W1:C32_W1 + 512])
        nc.vector.tensor_copy(out=u2h, in_=w2_f)
        nc.vector.tensor_copy(out=ugv, in_=gamma)
        nc.vector.memset(m1s, 0.0)
        nc.vector.memset(m2s, 0.0)
        nc.vector.memset(mgv, 0.0)

        # -------- DRAM staging (chunk-indexed) --------
        kc_st = dstage.tile([64, NCH, 128], F16)
        dhh_st = dstage.tile([64, NCH, 128], F16)
        dhpre_st = dstage.tile([64, NCH, 512], F16)
        hact_st = dstage.tile([64, NCH, 512], F16)
        q_st = dstage.tile([128, NCH, CHUNK], F16)
        g_st = dstage.tile([128, NCH, CHUNK], F16)
        md_st = dstage.tile([128, NCH, 4], F32)

        # ================= PHASE A: store-side, streamed per token-tile ========
        with tc.tile_pool(name="phA", bufs=1) as pA, \
             tc.tile_pool(name="psA", bufs=1, space="PSUM") as psA:
            # w2T (dh, hid) from w2 tiles via PE transpose
            for j in range(4):
                tp_ps = psA.tile([128, 128], F16, tag="tp", bufs=2)
                nc.tensor.transpose(tp_ps, w2_h[:, ts(j, 128)], ident_h)
                nc.vector.tensor_copy(out=w2T_h[:, ts(j, 128)], in_=tp_ps)

            for tt in range(NTT):
                tsl = ts(tt, 512)
                csl = slice(tt * 8, (tt + 1) * 8)
                seq_t = pA.tile([128, 4, 512], F16, tag="seq_t", bufs=2)
                dma(seq_t,
                    d["seqT"].ap().rearrange("(a p) t -> p a t", p=128)[:, :, tsl])
                # rmsnorm scale
                ss_ps = psA.tile([1, 512], F32, tag="mix", bufs=2)
                for j in range(4):
                    sqs = pA.tile([128, 512], F16, tag="sqs", bufs=2)
                    nc.scalar.activation(out=sqs, in_=seq_t[:, j, :], func=AF.Square)
                    nc.tensor.matmul(ss_ps, ones_col_h, sqs,
                                     start=(j == 0), stop=(j == 3))
                rowt = pA.tile([1, 512], F32, tag="rows", bufs=10)
                nc.scalar.activation(out=rowt, in_=ss_ps, func=AF.Sqrt,
                                     scale=1.0 / DIM, bias=epsT[0:1, :])
                rs_f = pA.tile([1, 512], F32, tag="rows", bufs=10)
                nc.vector.reciprocal(out=rs_f, in_=rowt)
                rs_h = pA.tile([1, 512], F16, tag="rows", bufs=10)
                nc.scalar.copy(out=rs_h, in_=rs_f)
                rsb_ps = psA.tile([128, 512], F32, tag="bc", bufs=2)
                nc.tensor.matmul(rsb_ps, ones_row_h, rs_h, start=True, stop=True)
                sT_t = pA.tile([128, 4, 512], F16, tag="sT_t", bufs=2)
                for j in range(4):
                    nc.vector.tensor_mul(out=sT_t[:, j, :], in0=seq_t[:, j, :],
                                         in1=rsb_ps)

                # projections
                k_ps = psA.tile([128, 512], F32, tag="proj", bufs=2)
                for j in range(4):
                    nc.tensor.matmul(k_ps, wk_h[:, ts(j, 128)], sT_t[:, j, :],
                                     start=(j == 0), stop=(j == 3))
                kT_r = pA.tile([128, 512], F32R, tag="kT_r")
                nc.vector.tensor_copy(out=kT_r, in_=k_ps)
                kT_h = pA.tile([128, 512], F16, tag="kT_h")
                nc.scalar.copy(out=kT_h, in_=k_ps)
                v_ps = psA.tile([128, 512], F32, tag="proj", bufs=2)
                for j in range(4):
                    nc.tensor.matmul(v_ps, wv_h[:, ts(j, 128)], sT_t[:, j, :],
                                     start=(j == 0), stop=(j == 3))
                kvT = pA.tile([128, 512], F32, tag="kvT")
                nc.vector.tensor_sub(out=kvT, in0=kT_r.bitcast(F32), in1=v_ps)
                q_ps = psA.tile([128, 512], F32, tag="proj", bufs=2)
                for j in range(4):
                    nc.tensor.matmul(q_ps, wq_h[:, ts(j, 128)], sT_t[:, j, :],
                                     start=(j == 0), stop=(j == 3))
                q_h = pA.tile([128, 512], F16, tag="q_h", bufs=2)
                nc.scalar.copy(out=q_h, in_=q_ps)
                dma(q_st[:, csl, :], q_h.rearrange("p (c k) -> p c k", k=CHUNK))
                sm_ps = psA.tile([4, 512], F32, tag="mix", bufs=2)
                for j in range(4):
                    nc.tensor.matmul(sm_ps, wsm_h[:, ts(j, 4)], sT_t[:, j, :],
                                     start=(j == 0), stop=(j == 3))
                # copy to sbuf, then extract rows at partition 0 via tiny DMAs
                smsb = pA.tile([4, 512], F32, tag="smsb", bufs=2)
                nc.vector.tensor_copy(out=smsb, in_=sm_ps)
                lr_row = pA.tile([1, 512], F32, tag="rows", bufs=10)
                gt_row = pA.tile([1, 512], F32, tag="rows", bufs=10)
                md_rows = pA.tile([2, 512], F32, tag="md_rows", bufs=2)
                dma(lr_row, smsb[0:1, :])
                dma(gt_row, smsb[3:4, :])
                dma(md_rows, smsb[1:3, :])
                nc.vector.tensor_reduce(
                    out=mdraw[:, csl],
                    in_=md_rows.rearrange("p (c k) -> p c k", k=CHUNK),
                    axis=X_AXIS, op=AX.add)
                lr_h = pA.tile([1, 512], F16, tag="rows", bufs=10)
                nc.scalar.copy(out=lr_h, in_=lr_row)
                gt_h = pA.tile([1, 512], F16, tag="rows", bufs=10)
                nc.scalar.copy(out=gt_h, in_=gt_row)
                lg_ps = psA.tile([128, 512], F32, tag="bc", bufs=2)
                nc.tensor.matmul(lg_ps, ones_row_h, lr_h, start=True, stop=True)
                lrB = pA.tile([128, 512], F32, tag="lrB")
                nc.scalar.activation(out=lrB, in_=lg_ps, func=AF.Sigmoid,
                                     bias=biasB[:, 0:1])
                gt_ps = psA.tile([128, 512], F32, tag="bc", bufs=2)
                nc.tensor.matmul(gt_ps, ones_row_h, gt_h, start=True, stop=True)
                gate_t = pA.tile([128, 512], F16, tag="gate_t", bufs=2)
                nc.scalar.activation(out=gate_t, in_=gt_ps, func=AF.Sigmoid)
                dma(g_st[:, csl, :], gate_t.rearrange("p (c k) -> p c k", k=CHUNK))

                # forward MLP (h_pre in fp32r, rest fp16)
                hact_h = pA.tile([128, 4, 512], F16, tag="hact_h")
                dgel = pA.tile([128, 4, 512], F32, tag="dgel")
                for j in range(4):
                    hp_ps = psA.tile([128, 512], F32, tag="proj", bufs=2)
                    nc.tensor.matmul(hp_ps, w1_r[:, ts(j, 128)], kT_r,
                                     start=True, stop=True)
                    nc.scalar.activation(out=hact_h[:, j, :], in_=hp_ps,
                                         func=AF.Gelu)
                    nc.scalar.activation(out=dgel[:, j, :], in_=hp_ps,
                                         func=AF.Derivative_Gelu)
                hh_ps = psA.tile([128, 512], F32, tag="proj", bufs=2)
                for j in range(4):
                    nc.tensor.matmul(hh_ps, w2_h[:, ts(j, 128)], hact_h[:, j, :],
                                     start=(j == 0), stop=(j == 3))
                hhsb = pA.tile([128, 512], F32, tag="hhsb")
                nc.vector.tensor_copy(out=hhsb, in_=hh_ps)
                sq2 = pA.tile([128, 512], F16, tag="sq2", bufs=2)
                nc.scalar.activation(out=sq2, in_=hh_ps, func=AF.Square)
                ms_ps = psA.tile([1, 512], F32, tag="mix", bufs=2)
                nc.tensor.matmul(ms_ps, ones_col_h, sq2, start=True, stop=True)
                rowt2 = pA.tile([1, 512], F32, tag="rows", bufs=10)
                nc.scalar.activation(out=rowt2, in_=ms_ps, func=AF.Sqrt,
                                     scale=1.0 / DH, bias=epsT[0:1, :])
                srs_f = pA.tile([1, 512], F32, tag="rows", bufs=10)
                nc.vector.reciprocal(out=srs_f, in_=rowt2)
                srs_h = pA.tile([1, 512], F16, tag="rows", bufs=10)
                nc.scalar.copy(out=srs_h, in_=srs_f)
                srsb_ps = psA.tile([128, 512], F32, tag="bc", bufs=2)
                nc.tensor.matmul(srsb_ps, ones_row_h, srs_h, start=True, stop=True)
                ysb = pA.tile([128, 512], F32, tag="ysb")
                nc.vector.tensor_mul(out=ysb, in0=hhsb, in1=srsb_ps)
                dp = pA.tile([128, 512], F32, tag="dp")
                nc.vector.scalar_tensor_tensor(out=dp, in0=ysb, scalar=gamma,
                                               in1=kvT, op0=AX.mult, op1=AX.add)
                nc.vector.tensor_mul(out=dp, in0=dp, in1=lrB)
                gp = pA.tile([128, 512], F32, tag="gp", bufs=2)
                nc.vector.tensor_mul(out=gp, in0=dp, in1=ysb)
                nc.vector.tensor_reduce(out=gG[:, csl],
                                        in_=gp.rearrange("p (c k) -> p c k", k=CHUNK),
                                        axis=X_AXIS, op=AX.add)
                dY = pA.tile([128, 512], F32, tag="dY")
                nc.vector.tensor_scalar_mul(out=dY, in0=dp, scalar1=gamma)
                dprod = pA.tile([128, 512], F16, tag="dprod", bufs=2)
                nc.vector.tensor_mul(out=dprod, in0=dY, in1=hhsb)
                dot_ps = psA.tile([1, 512], F32, tag="mix", bufs=2)
                nc.tensor.matmul(dot_ps, ones_col_h, dprod, start=True, stop=True)
                s3 = pA.tile([1, 512], F32, tag="rows", bufs=10)
                nc.vector.tensor_mul(out=s3, in0=srs_f, in1=srs_f)
                nc.vector.tensor_mul(out=s3, in0=s3, in1=srs_f)
                c_f = pA.tile([1, 512], F32, tag="rows", bufs=10)
                nc.vector.tensor_mul(out=c_f, in0=s3, in1=dot_ps)
                c_h = pA.tile([1, 512], F16, tag="rows", bufs=10)
                nc.scalar.activation(out=c_h, in_=c_f, func=AF.Copy, scale=1.0 / DH)
                cb_ps = psA.tile([128, 512], F32, tag="bc", bufs=2)
                nc.tensor.matmul(cb_ps, ones_row_h, c_h, start=True, stop=True)
                m1t = pA.tile([128, 512], F32, tag="m1t", bufs=2)
                nc.vector.tensor_mul(out=m1t, in0=dY, in1=srsb_ps)
                m2t = pA.tile([128, 512], F32, tag="m2t", bufs=2)
                nc.vector.tensor_mul(out=m2t, in0=hhsb, in1=cb_ps)
                dhh_h = pA.tile([128, 512], F16, tag="dhh_h")
                nc.vector.tensor_sub(out=dhh_h, in0=m1t, in1=m2t)

                # backward to dhpre (fp16)
                dhpre_h = pA.tile([128, 4, 512], F16, tag="dhpre_h")
                for j in range(4):
                    da_ps = psA.tile([128, 512], F32, tag="proj", bufs=2)
                    nc.tensor.matmul(da_ps, w2T_h[:, ts(j, 128)], dhh_h,
                                     start=True, stop=True)
                    nc.vector.tensor_mul(out=dhpre_h[:, j, :], in0=da_ps,
                                         in1=dgel[:, j, :])

                # token-major transposes (fp16) -> staging -> chunk-major DRAM
                st_kc = pA.tile([128, 4, 128], F16, tag="st_kc", bufs=1)
                st_dh = pA.tile([128, 4, 128], F16, tag="st_dh", bufs=1)
                st_dp = pA.tile([128, 4, 512], F16, tag="st_dp", bufs=1)
                st_ha = pA.tile([128, 4, 512], F16, tag="st_ha", bufs=1)
                for blk in range(4):
                    bsl = ts(blk, 128)
                    tp_ps = psA.tile([128, 4, 128], F16, tag="tp", bufs=2)
                    nc.tensor.transpose(tp_ps[:, 0, :], kT_h[:, bsl], ident_h)
                    nc.tensor.transpose(tp_ps[:, 1, :], dhh_h[:, bsl], ident_h)
                    nc.vector.tensor_copy(out=st_kc[:, blk, :], in_=tp_ps[:, 0, :])
                    nc.vector.tensor_copy(out=st_dh[:, blk, :], in_=tp_ps[:, 1, :])
                    for j in range(4):
                        t2_ps = psA.tile([128, 4, 128], F16, tag="tp", bufs=2)
                        nc.tensor.transpose(t2_ps[:, 0, :], dhpre_h[:, j, bsl],
                                            ident_h)
                        nc.tensor.transpose(t2_ps[:, 1, :], hact_h[:, j, bsl],
                                            ident_h)
                        nc.vector.tensor_copy(out=st_dp[:, blk, ts(j, 128)],
                                              in_=t2_ps[:, 0, :])
                        nc.vector.tensor_copy(out=st_ha[:, blk, ts(j, 128)],
                                              in_=t2_ps[:, 1, :])
                for cm, stg in [(kc_st, st_kc), (dhh_st, st_dh),
                                (dhpre_st, st_dp), (hact_st, st_ha)]:
                    v = cm.rearrange("p (a two) x -> p a two x", two=2)
                    dma(v[:, 4 * tt:4 * tt + 4, 0, :], stg[0:64, :, :])
                    dma(v[:, 4 * tt:4 * tt + 4, 1, :], stg[64:128, :, :])

            # finish mom/dec (all chunks) -> md_st = [mom, 1-dec, gG, pad]
            mds = pA.tile([2, NCH], F32, tag="mds")
            nc.scalar.activation(out=mds, in_=mdraw, func=AF.Sigmoid,
                                 scale=1.0 / CHUNK, bias=bias_md)
            mrow_f = pA.tile([1, NCH], F32, tag="mrow_f")
            drow_f = pA.tile([1, NCH], F32, tag="drow_f")
            dma(mrow_f, mds[0:1, :])
            dma(drow_f, mds[1:2, :])
            mrow = pA.tile([1, NCH], F16, tag="mrow")
            drow = pA.tile([1, NCH], F16, tag="drow")
            nc.scalar.copy(out=mrow, in_=mrow_f)
            nc.scalar.copy(out=drow, in_=drow_f)
            mb_ps = psA.tile([128, 512], F32, tag="bc", bufs=2)
            nc.tensor.matmul(mb_ps[:, 0:NCH], ones_row_h, mrow, start=True, stop=True)
            nc.tensor.matmul(mb_ps[:, 64:64 + NCH], ones_row_h, drow,
                             start=True, stop=True)
            momB = pA.tile([128, NCH], F32, tag="momB")
            nc.vector.tensor_copy(out=momB, in_=mb_ps[:, 0:NCH])
            decm1B = pA.tile([128, NCH], F32, tag="decm1B")
            nc.scalar.activation(out=decm1B, in_=mb_ps[:, 64:64 + NCH],
                                 func=AF.Identity, scale=-1.0, bias=1.0)
            nc.vector.tensor_scalar_mul(out=gG, in0=gG, scalar1=-2.0 / DH)
            mdv = md_st.rearrange("p c x -> p x c")
            dma(mdv[:, 0, :], momB)
            dma(mdv[:, 1, :], decm1B)
            dma(mdv[:, 2, :], gG)

        # ============ CHUNK LOOP: retrieval + grads + NS5 + scans ============
        with tc.tile_pool(name="phL", bufs=1) as pL, \
             tc.tile_pool(name="psL", bufs=1, space="PSUM") as psL:
            with tc.For_i(0, NCH, 1) as c:
                kc_t = pL.tile([64, 128], F16, tag="kc_t", bufs=2)
                dma(kc_t, kc_st[:, ds(c, 1), :].rearrange("p one x -> p (one x)"))
                dhh_t = pL.tile([64, 128], F16, tag="dhh_t", bufs=2)
                dma(dhh_t, dhh_st[:, ds(c, 1), :].rearrange("p one x -> p (one x)"))
                dhpre_t = pL.tile([64, 512], F16, tag="dhpre_t", bufs=2)
                dma(dhpre_t,
                    dhpre_st[:, ds(c, 1), :].rearrange("p one x -> p (one x)"))
                hact_t = pL.tile([64, 512], F16, tag="hact_t", bufs=2)
                dma(hact_t,
                    hact_st[:, ds(c, 1), :].rearrange("p one x -> p (one x)"))
                q_t = pL.tile([128, CHUNK], F16, tag="q_t", bufs=2)
                dma(q_t, q_st[:, ds(c, 1), :].rearrange("p one x -> p (one x)"))
                gate_t = pL.tile([128, CHUNK], F16, tag="gate_t", bufs=2)
                dma(gate_t, g_st[:, ds(c, 1), :].rearrange("p one x -> p (one x)"))
                md_t = pL.tile([128, 4], F32, tag="md_t", bufs=2)
                dma(md_t, md_st[:, ds(c, 1), :].rearrange("p one x -> p (one x)"))

                # ---- retrieval with pre-update state ----
                hp_ps = psL.tile([128, 4, CHUNK], F32, tag="pr", bufs=1)
                for j in range(4):
                    nc.tensor.matmul(hp_ps[:, j, :], u1h[:, ts(j, 128)], q_t,
                                     start=True, stop=True)
                ha_c = pL.tile([128, 4, CHUNK], F16, tag="ha_c", bufs=2)
                nc.scalar.activation(out=ha_c, in_=hp_ps, func=AF.Gelu)
                hh_ps = psL.tile([128, CHUNK], F32, tag="pr", bufs=1)
                for j in range(4):
                    nc.tensor.matmul(hh_ps, u2h[:, ts(j, 128)], ha_c[:, j, :],
                                     start=(j == 0), stop=(j == 3))
                sqc = pL.tile([128, CHUNK], F16, tag="sqc", bufs=2)
                nc.scalar.activation(out=sqc, in_=hh_ps, func=AF.Square)
                hhc = pL.tile([128, CHUNK], F32, tag="hhc", bufs=2)
                nc.scalar.copy(out=hhc, in_=hh_ps)
                ms_ps = psL.tile([1, CHUNK], F32, tag="prow", bufs=1)
                nc.tensor.matmul(ms_ps, ones_col_h, sqc, start=True, stop=True)
                rr = pL.tile([1, CHUNK], F32, tag="rr", bufs=2)
                nc.scalar.activation(out=rr, in_=ms_ps, func=AF.Sqrt,
                                     scale=1.0 / DH, bias=epsT[0:1, :])
                rr2 = pL.tile([1, CHUNK], F32, tag="rr2", bufs=2)
                nc.vector.reciprocal(out=rr2, in_=rr)
                rrh = pL.tile([1, CHUNK], F16, tag="rrh", bufs=2)
                nc.scalar.copy(out=rrh, in_=rr2)
                sb_ps = psL.tile([128, CHUNK], F32, tag="pr", bufs=1)
                nc.tensor.matmul(sb_ps, ones_row_h, rrh, start=True, stop=True)
                yc = pL.tile([128, CHUNK], F32, tag="yc", bufs=2)
                nc.vector.tensor_mul(out=yc, in0=hhc, in1=sb_ps)
                prc = pL.tile([128, CHUNK], F32, tag="prc", bufs=2)
                nc.vector.scalar_tensor_tensor(out=prc, in0=yc, scalar=ugv,
                                               in1=q_t, op0=AX.mult, op1=AX.add)
                outc = pL.tile([128, CHUNK], F16, tag="outc", bufs=2)
                nc.vector.tensor_mul(out=outc, in0=prc, in1=gate_t)
                dma(d["out"].ap()[ds(c, 1)].rearrange("one p x -> (one p) x"),
                    outc)

                # ---- inner grads (g1 = dL/dw1, g2t = (dL/dw2)^T, both [dh,hid])
                g1_ps = psL.tile([128, 512], F32, tag="pg", bufs=2)
                nc.tensor.matmul(g1_ps, kc_t, dhpre_t, start=True, stop=True)
                g2_ps = psL.tile([128, 512], F32, tag="pg", bufs=2)
                nc.tensor.matmul(g2_ps, dhh_t, hact_t, start=True, stop=True)
                g1T_ps = psL.tile([128, 4, 128], F32, tag="pgT", bufs=2)
                for j in range(4):
                    nc.tensor.matmul(g1T_ps[:, j, :], dhpre_t[:, ts(j, 128)],
                                     kc_t, start=True, stop=True)
                g2T_ps = psL.tile([128, 4, 128], F32, tag="pgT", bufs=2)
                for j in range(4):
                    nc.tensor.matmul(g2T_ps[:, j, :], hact_t[:, ts(j, 128)],
                                     dhh_t, start=True, stop=True)
                g1sb = pL.tile([128, 512], F32R, tag="g1sb", bufs=2)
                nc.vector.tensor_copy(out=g1sb, in_=g1_ps)
                g2sb = pL.tile([128, 512], F32R, tag="g2sb", bufs=2)
                nc.vector.tensor_copy(out=g2sb, in_=g2_ps)
                R = pL.tile([128, 2], F32, tag="R", bufs=2)
                scr = pL.tile([128, 512], F16, tag="scr", bufs=2)
                nc.vector.scalar_tensor_tensor(
                    out=scr, in0=g1sb.bitcast(F32), scalar=1.0,
                    in1=g1sb.bitcast(F32), op0=AX.mult, op1=AX.mult,
                    accum_out=R[:, 0:1])
                scr2 = pL.tile([128, 512], F16, tag="scr", bufs=2)
                nc.vector.scalar_tensor_tensor(
                    out=scr2, in0=g2sb.bitcast(F32), scalar=1.0,
                    in1=g2sb.bitcast(F32), op0=AX.mult, op1=AX.mult,
                    accum_out=R[:, 1:2])
                Rh = pL.tile([128, 2], F16, tag="Rh", bufs=2)
                nc.vector.tensor_copy(out=Rh, in_=R)
                nrm_ps = psL.tile([1, 2], F32, tag="prow", bufs=1)
                nc.tensor.matmul(nrm_ps, ones_col_h, Rh, start=True, stop=True)
                inv2 = pL.tile([1, 2], F32, tag="inv2", bufs=2)
                nc.vector.reciprocal(out=inv2, in_=nrm_ps)
                ninv = pL.tile([1, 2], F32, tag="ninv", bufs=2)
                nc.scalar.activation(out=ninv, in_=inv2, func=AF.Sqrt)
                nc.scalar.activation(out=ninv, in_=ninv, func=AF.Copy, scale=-1.0)
                nb = pL.tile([128, 2], F32, tag="nb", bufs=2)
                nc.gpsimd.partition_broadcast(nb, ninv)

                # ---- NS5, fp32 (f32r matmuls), transpose-free ----
                tP = [None, None]
                tT = [None, None]
                tP[0] = pL.tile([128, 512], F32R, tag="tPa", bufs=2, name="tP0")
                nc.vector.tensor_scalar_mul(out=tP[0], in0=g1sb.bitcast(F32),
                                            scalar1=nb[:, 0:1])
                tT[0] = pL.tile([128, 4, 128], F32R, tag="tTa", bufs=2, name="tT0")
                nc.vector.tensor_scalar_mul(out=tT[0], in0=g1T_ps,
                                            scalar1=nb[:, 0:1])
                tP[1] = pL.tile([128, 512], F32R, tag="tPb", bufs=2, name="tP1")
                nc.vector.tensor_scalar_mul(out=tP[1], in0=g2sb.bitcast(F32),
                                            scalar1=nb[:, 1:2])
                tT[1] = pL.tile([128, 4, 128], F32R, tag="tTb", bufs=2, name="tT1")
                nc.vector.tensor_scalar_mul(out=tT[1], in0=g2T_ps,
                                            scalar1=nb[:, 1:2])

                for k in range(5):
                    last = k == 4
                    for i in range(2):
                        A_ps = psL.tile([128, 128], F32, tag="pA", bufs=2)
                        for j in range(4):
                            nc.tensor.matmul(A_ps, tT[i][:, j, :], tT[i][:, j, :],
                                             start=(j == 0), stop=(j == 3))
                        Ab = pL.tile([128, 128], F32R, tag="Ab", bufs=2)
                        nc.vector.tensor_scalar_mul(out=Ab, in0=A_ps, scalar1=NSB)
                        Au = pL.tile([128, 128], F32R, tag="Au", bufs=2)
                        nc.scalar.copy(out=Au, in_=A_ps)
                        A2_ps = psL.tile([128, 128], F32, tag="pA", bufs=2)
                        nc.tensor.matmul(A2_ps, Ab, Au, start=True, stop=False)
                        nc.tensor.matmul(A2_ps, identr, aIc, start=False, stop=True)
                        Bm = pL.tile([128, 128], F32R, tag="Bm", bufs=2)
                        nc.vector.scalar_tensor_tensor(
                            out=Bm, in0=A2_ps, scalar=NSC / NSB,
                            in1=Ab.bitcast(F32), op0=AX.mult, op1=AX.add)
                        if not (last and i == 1):
                            tp_ps = psL.tile([128, 512], F32, tag="pg", bufs=2)
                            nc.tensor.matmul(tp_ps, Bm, tP[i], start=True,
                                             stop=True)
                            tPn = pL.tile([128, 512], F32R,
                                          tag=("tPa" if i == 0 else "tPb"), bufs=2)
                            nc.scalar.copy(out=tPn, in_=tp_ps)
                        else:
                            tPn = tP[i]
                        if not (last and i == 0):
                            tt_ps = psL.tile([128, 4, 128], F32, tag="pgT", bufs=2)
                            for j in range(4):
                                nc.tensor.matmul(tt_ps[:, j, :],
                                                 tP[i][:, ts(j, 128)], Bm,
                                                 start=True, stop=True)
                            tTn = pL.tile([128, 4, 128], F32R,
                                          tag=("tTa" if i == 0 else "tTb"), bufs=2)
                            nc.scalar.copy(out=tTn, in_=tt_ps)
                        else:
                            tTn = tT[i]
                        tP[i] = tPn
                        tT[i] = tTn

                s1 = tP[0].bitcast(F32)
                s2 = tT[1].rearrange("p a b -> p (a b)").bitcast(F32)

                # ---- scans (momentum then weight-decay), fp32 ----
                nc.vector.scalar_tensor_tensor(out=m1s, in0=m1s,
                                               scalar=md_t[:, 0:1], in1=s1,
                                               op0=AX.mult, op1=AX.add)
                nc.vector.scalar_tensor_tensor(out=u1, in0=u1,
                                               scalar=md_t[:, 1:2], in1=m1s,
                                               op0=AX.mult, op1=AX.add)
                nc.scalar.copy(out=u1h, in_=u1)
                nc.vector.scalar_tensor_tensor(out=m2s, in0=m2s,
                                               scalar=md_t[:, 0:1], in1=s2,
                                               op0=AX.mult, op1=AX.add)
                nc.vector.scalar_tensor_tensor(out=u2, in0=u2,
                                               scalar=md_t[:, 1:2], in1=m2s,
                                               op0=AX.mult, op1=AX.add)
                nc.scalar.copy(out=u2h, in_=u2)
                nc.vector.scalar_tensor_tensor(out=mgv, in0=mgv,
                                               scalar=md_t[:, 0:1],
                                               in1=md_t[:, 2:3],
                                               op0=AX.mult, op1=AX.add)
                nc.vector.scalar_tensor_tensor(out=ugv, in0=ugv,
                                               scalar=md_t[:, 1:2], in1=mgv,
                                               op0=AX.mult, op1=AX.add)


# ------------------- host side -------------------

def _prep_core_inputs(inputs, b, h):
    f = np.float32
    sg = np.asarray(inputs["store_g"], f)[:, None]
    rg = np.asarray(inputs["retrieve_g"], f)[:, None]
    hs = slice(h * DH, (h + 1) * DH)

    def tile128(w):  # (512, X) -> rows grouped as (128, 4, X) -> (128, 4*X)
        w = np.asarray(w, f)
        return np.ascontiguousarray(
            w.reshape(4, 128, -1).transpose(1, 0, 2).reshape(128, -1))

    wk = tile128(sg * np.asarray(inputs["Wk"], f)[:, hs])
    wv = tile128(sg * np.asarray(inputs["Wv"], f)[:, hs])
    wq = tile128(rg * np.asarray(inputs["Wq"], f)[:, hs])
    wsm = tile128(np.stack([
        sg[:, 0] * np.asarray(inputs["W_lr"], f)[:, h],
        sg[:, 0] * np.asarray(inputs["Wm"], f)[:, h],
        sg[:, 0] * np.asarray(inputs["Wd"], f)[:, h],
        rg[:, 0] * np.asarray(inputs["Wgate"], f)[:, h]], axis=1))
    ident = np.eye(128, dtype=f)
    cw16 = np.concatenate([wk, wv, wq, wsm, ident], axis=1).astype(np.float16)

    w1 = np.asarray(inputs["mw1"], f)[h]
    w2 = tile128(np.asarray(inputs["mw2"], f)[h])
    gamma = np.asarray(inputs["mgamma"], f)[h].reshape(128, 1)
    biasB = np.broadcast_to(
        np.array([inputs["b_lr"][h], 0.0, 0.0, 0.0], f), (128, 4))
    mdcol = np.zeros((128, 1), f)
    mdcol[0, 0] = inputs["bm"][h]
    mdcol[1, 0] = inputs["bd"][h]
    cw32 = np.ascontiguousarray(
        np.concatenate([w1, w2, gamma, biasB, mdcol], axis=1), f)

    seqT = np.asarray(inputs["seq"], f)[b].T.astype(np.float16)
    return {"seqT": np.ascontiguousarray(seqT), "cw16": cw16, "cw32": cw32}


_CACHE = {}


def _get_module():
    if "nc" not in _CACHE:
        nc = bacc.Bacc("TRN2", target_bir_lowering=False, debug=False)
        build(nc)
        nc.compile()
        _CACHE["nc"] = nc
    return _CACHE["nc"]


def kernel(**inputs):
    from concourse.bass_utils import run_bass_kernel_spmd
    nc = _get_module()
    in_maps = [_prep_core_inputs(inputs, core // HEADS, core % HEADS)
               for core in range(8)]
    res = run_bass_kernel_spmd(nc, in_maps, core_ids=list(range(8)))
    _CACHE["last_res"] = res
    Wc = np.asarray(inputs["Wc"], np.float32)
    out = np.zeros((B, N, DIM), np.float32)
    for b in range(B):
        heads = []
        for h in range(HEADS):
            oc = res.results[b * HEADS + h]["out"]  # (NCH, 128, CHUNK) f16
            heads.append(
                oc.transpose(0, 2, 1).reshape(N, DH).astype(np.float32))
        out[b] = np.concatenate(heads, axis=1) @ Wc
    return out


if __name__ == "__main__":
    dd = np.load("/root/problem/ref_inputs.npz")
    inputs = {k: dd[k] for k in dd.files}
    out = kernel(**inputs)
    exp = np.load("/root/problem/ref_expected.npy")
    err = np.abs(out - exp).max() / np.abs(exp).max()
    rel = np.linalg.norm(out - exp) / np.linalg.norm(exp)
    print(f"absmax-rel: {err:.3e}  l2-rel: {rel:.3e}")


# revision 11
# speedup vs baseline: 4.0204x; 1.1407x over previous
"""Trainium2 Bass kernel for nn_NeuralMemory (Titans-style neural memory).

Sharding: 8 cores <-> 8 (batch, head) pairs. Each core runs the full
per-(b,h) pipeline; the host applies the final Wc projection and sums
the 4 head partials per batch (268 MFLOP of BLAS, ~ms).

This revision optimizes END-TO-END dispatch cost, which dominates the
measured time on the axon/PJRT path (per-call jit+compile-cache+load
scales with program size; transfers scale with I/O bytes):
  - I/O contract shrunk ~4x: fp16 seq, packed fp16/fp32 weight tensors,
    fp16 [NCH,128,64] gated output (pre-Wc); dead inputs dropped.
  - Instruction count ~25x smaller: the 32-chunk pipeline (inner grads,
    NS5, momentum/decay scans, retrieval) runs in ONE hardware For_i
    loop instead of being fully unrolled.
  - Newton-Schulz-5 runs in fp32 (f32r matmuls) — device time is cheap
    here, and it cuts rel-err ~15x vs the old fp16 NS (1e-3 vs 1.6e-2).
    Transpose-free NS: maintain t [dh,hid] and tT blocks; with
    Bm' = aI + bA + cA^2 (A = t t^T, symmetric), t_new = Bm' t and
    tT_new_j = t_j^T Bm' are plain matmuls off the OLD t — no PE
    transposes inside the iteration.

Math restructuring (validated vs the jax reference in numpy at ~1.4e-5
fp32 / ~1.0e-3 with fp16 phase A):
  - rmsnorm gains folded into projection weights (host-side).
  - inner-loss grads derived manually at the shared initial fast
    weights; the 2/DH*lr factor is dropped for g1/g2 (NS is
    scale-invariant) and applied only to the gamma grad.
  - momentum/decay scans fused per chunk with retrieval (which uses the
    weights from the end of the previous chunk).
"""
import sys

sys.path.insert(0, "/opt/trn_rl_repo")

import numpy as np

import concourse.bass as bass
import concourse.bacc as bacc
import concourse.mybir as mybir
import concourse.tile as tile
from concourse.bass import ts, ds

F32 = mybir.dt.float32
F32R = mybir.dt.float32r
F16 = mybir.dt.float16

DIM, HEADS, DH, CHUNK = 512, 4, 128, 64
HID = DH * 4
B, N = 2, 2048
NCH = N // CHUNK          # 32 chunks
NTT = N // 512            # 4 token tiles
NSA, NSB, NSC = 3.4445, -4.775, 2.0315
AX = mybir.AluOpType
AF = mybir.ActivationFunctionType
X_AXIS = mybir.AxisListType.X

# packed fp16 const columns: wk | wv | wq | wsm | ident | w1 | w2
C16_WK, C16_WV, C16_WQ, C16_WSM, C16_ID = 0, 512, 1024, 1536, 1552
C16_W1, C16_W2 = 1680, 2192
K16 = 2192 + 512
# packed fp32 const columns: gamma | biasB | md-bias col
C32_G, C32_BB, C32_MD = 0, 1, 5
K32 = 6


def build(nc):
    d = {}
    d["seqT"] = nc.dram_tensor("seqT", [DIM, N], F16, kind="ExternalInput")
    d["cw16"] = nc.dram_tensor("cw16", [128, K16], F16, kind="ExternalInput")
    d["cw32"] = nc.dram_tensor("cw32", [128, K32], F32, kind="ExternalInput")
    d["out"] = nc.dram_tensor("out", [NCH, 128, CHUNK], F16, kind="ExternalOutput")
    with tile.TileContext(nc) as tc:
        _body(nc, tc, d)
    return nc


def _body(nc, tc, d):
    def dma(out, in_):
        nc.sync.dma_start(out=out, in_=in_)

    consts_cm = tc.tile_pool(name="consts", bufs=1)
    persist_cm = tc.tile_pool(name="persist", bufs=1)
    dram_cm = tc.tile_pool(name="dstage", bufs=1, space="DRAM")
    with consts_cm as consts, persist_cm as persist, dram_cm as dstage:
        # ---------------- constants ----------------
        cw16 = consts.tile([128, K16], F16)
        dma(cw16, d["cw16"].ap())
        cw32 = consts.tile([128, K32], F32)
        dma(cw32, d["cw32"].ap())
        wk_h = cw16[:, C16_WK:C16_WK + 512]
        wv_h = cw16[:, C16_WV:C16_WV + 512]
        wq_h = cw16[:, C16_WQ:C16_WQ + 512]
        wsm_h = cw16[:, C16_WSM:C16_WSM + 16]
        ident_h = cw16[:, C16_ID:C16_ID + 128]
        gamma = cw32[:, C32_G:C32_G + 1]
        biasB = cw32[:, C32_BB:C32_BB + 4]
        bias_md = cw32[0:2, C32_MD:C32_MD + 1]

        epsT = consts.tile([128, 1], F32)
        nc.vector.memset(epsT, 1e-6)
        ones_col_h = consts.tile([128, 1], F16)
        nc.vector.memset(ones_col_h, 1.0)
        ones_row_h = consts.tile([1, 128], F16)
        nc.vector.memset(ones_row_h, 1.0)
        identr = consts.tile([128, 128], F32R)
        nc.vector.tensor_copy(out=identr, in_=ident_h)
        aIc = consts.tile([128, 128], F32R)
        nc.scalar.activation(out=aIc, in_=identr.bitcast(F32), func=AF.Copy,
                             scale=NSA * NSB / NSC)
        w1_h = cw16[:, C16_W1:C16_W1 + 512]
        w2_h = cw16[:, C16_W2:C16_W2 + 512]
        w1_r = consts.tile([128, 512], F32R)
        nc.vector.tensor_copy(out=w1_r, in_=w1_h)

        # -------- persistent state --------
        u1 = persist.tile([128, 512], F32)
        u2 = persist.tile([128, 512], F32)
        u1h = persist.tile([128, 512], F16)
        u2h = persist.tile([128, 512], F16)
        m1s = persist.tile([128, 512], F32)
        m2s = persist.tile([128, 512], F32)
        ugv = persist.tile([128, 1], F32)
        mgv = persist.tile([128, 1], F32)
        mdraw = persist.tile([2, NCH], F32)
        gG = persist.tile([128, NCH], F32)
        w2T_h = persist.tile([128, 512], F16)
        nc.vector.tensor_copy(out=u1, in_=w1_h)
        nc.vector.tensor_copy(out=u2, in_=w2_h)
        nc.vector.tensor_copy(out=u1h, in_=w1_h)
        nc.vector.tensor_copy(out=u2h, in_=w2_h)
        nc.vector.tensor_copy(out=ugv, in_=gamma)
        nc.vector.memset(m1s, 0.0)
        nc.vector.memset(m2s, 0.0)
        nc.vector.memset(mgv, 0.0)

        # -------- DRAM staging (chunk-indexed) --------
        kc_st = dstage.tile([64, NCH, 128], F16)
        dhh_st = dstage.tile([64, NCH, 128], F16)
        dhpre_st = dstage.tile([64, NCH, 512], F16)
        hact_st = dstage.tile([64, NCH, 512], F16)
        q_st = dstage.tile([128, NCH, CHUNK], F16)
        g_st = dstage.tile([128, NCH, CHUNK], F16)
        md_st = dstage.tile([128, NCH, 4], F32)

        # ================= PHASE A: store-side, streamed per token-tile ========
        with tc.tile_pool(name="phA", bufs=1) as pA, \
             tc.tile_pool(name="psA", bufs=1, space="PSUM") as psA:
            # w2T (dh, hid) from w2 tiles via PE transpose
            for j in range(4):
                tp_ps = psA.tile([128, 128], F16, tag="tp", bufs=2)
                nc.tensor.transpose(tp_ps, w2_h[:, ts(j, 128)], ident_h)
                nc.vector.tensor_copy(out=w2T_h[:, ts(j, 128)], in_=tp_ps)

            for tt in range(NTT):
                tsl = ts(tt, 512)
                csl = slice(tt * 8, (tt + 1) * 8)
                seq_t = pA.tile([128, 4, 512], F16, tag="seq_t", bufs=2)
                dma(seq_t,
                    d["seqT"].ap().rearrange("(a p) t -> p a t", p=128)[:, :, tsl])
                # rmsnorm scale
                ss_ps = psA.tile([1, 512], F32, tag="mix", bufs=2)
                for j in range(4):
                    sqs = pA.tile([128, 512], F16, tag="sqs", bufs=2)
                    nc.scalar.activation(out=sqs, in_=seq_t[:, j, :], func=AF.Square)
                    nc.tensor.matmul(ss_ps, ones_col_h, sqs,
                                     start=(j == 0), stop=(j == 3))
                rowt = pA.tile([1, 512], F32, tag="rows", bufs=10)
                nc.scalar.activation(out=rowt, in_=ss_ps, func=AF.Sqrt,
                                     scale=1.0 / DIM, bias=epsT[0:1, :])
                rs_f = pA.tile([1, 512], F32, tag="rows", bufs=10)
                nc.vector.reciprocal(out=rs_f, in_=rowt)
                rs_h = pA.tile([1, 512], F16, tag="rows", bufs=10)
                nc.scalar.copy(out=rs_h, in_=rs_f)
                rsb_ps = psA.tile([128, 512], F32, tag="bc", bufs=2)
                nc.tensor.matmul(rsb_ps, ones_row_h, rs_h, start=True, stop=True)
                sT_t = pA.tile([128, 4, 512], F16, tag="sT_t", bufs=2)
                for j in range(4):
                    nc.vector.tensor_mul(out=sT_t[:, j, :], in0=seq_t[:, j, :],
                                         in1=rsb_ps)

                # projections
                k_ps = psA.tile([128, 512], F32, tag="proj", bufs=2)
                for j in range(4):
                    nc.tensor.matmul(k_ps, wk_h[:, ts(j, 128)], sT_t[:, j, :],
                                     start=(j == 0), stop=(j == 3))
                kT_r = pA.tile([128, 512], F32R, tag="kT_r")
                nc.vector.tensor_copy(out=kT_r, in_=k_ps)
                kT_h = pA.tile([128, 512], F16, tag="kT_h")
                nc.scalar.copy(out=kT_h, in_=k_ps)
                v_ps = psA.tile([128, 512], F32, tag="proj", bufs=2)
                for j in range(4):
                    nc.tensor.matmul(v_ps, wv_h[:, ts(j, 128)], sT_t[:, j, :],
                                     start=(j == 0), stop=(j == 3))
                kvT = pA.tile([128, 512], F32, tag="kvT")
                nc.vector.tensor_sub(out=kvT, in0=kT_r.bitcast(F32), in1=v_ps)
                q_ps = psA.tile([128, 512], F32, tag="proj", bufs=2)
                for j in range(4):
                    nc.tensor.matmul(q_ps, wq_h[:, ts(j, 128)], sT_t[:, j, :],
                                     start=(j == 0), stop=(j == 3))
                q_h = pA.tile([128, 512], F16, tag="q_h", bufs=2)
                nc.scalar.copy(out=q_h, in_=q_ps)
                dma(q_st[:, csl, :], q_h.rearrange("p (c k) -> p c k", k=CHUNK))
                sm_ps = psA.tile([4, 512], F32, tag="mix", bufs=2)
                for j in range(4):
                    nc.tensor.matmul(sm_ps, wsm_h[:, ts(j, 4)], sT_t[:, j, :],
                                     start=(j == 0), stop=(j == 3))
                # copy to sbuf, then extract rows at partition 0 via tiny DMAs
                smsb = pA.tile([4, 512], F32, tag="smsb", bufs=2)
                nc.vector.tensor_copy(out=smsb, in_=sm_ps)
                lr_row = pA.tile([1, 512], F32, tag="rows", bufs=10)
                gt_row = pA.tile([1, 512], F32, tag="rows", bufs=10)
                md_rows = pA.tile([2, 512], F32, tag="md_rows", bufs=2)
                dma(lr_row, smsb[0:1, :])
                dma(gt_row, smsb[3:4, :])
                dma(md_rows, smsb[1:3, :])
                nc.vector.tensor_reduce(
                    out=mdraw[:, csl],
                    in_=md_rows.rearrange("p (c k) -> p c k", k=CHUNK),
                    axis=X_AXIS, op=AX.add)
                lr_h = pA.tile([1, 512], F16, tag="rows", bufs=10)
                nc.scalar.copy(out=lr_h, in_=lr_row)
                gt_h = pA.tile([1, 512], F16, tag="rows", bufs=10)
                nc.scalar.copy(out=gt_h, in_=gt_row)
                lg_ps = psA.tile([128, 512], F32, tag="bc", bufs=2)
                nc.tensor.matmul(lg_ps, ones_row_h, lr_h, start=True, stop=True)
                lrB = pA.tile([128, 512], F32, tag="lrB")
                nc.scalar.activation(out=lrB, in_=lg_ps, func=AF.Sigmoid,
                                     bias=biasB[:, 0:1])
                gt_ps = psA.tile([128, 512], F32, tag="bc", bufs=2)
                nc.tensor.matmul(gt_ps, ones_row_h, gt_h, start=True, stop=True)
                gate_t = pA.tile([128, 512], F16, tag="gate_t", bufs=2)
                nc.scalar.activation(out=gate_t, in_=gt_ps, func=AF.Sigmoid)
                dma(g_st[:, csl, :], gate_t.rearrange("p (c k) -> p c k", k=CHUNK))

                # forward MLP (h_pre in fp32r, rest fp16)
                hact_h = pA.tile([128, 4, 512], F16, tag="hact_h")
                dgel = pA.tile([128, 4, 512], F32, tag="dgel")
                for j in range(4):
                    hp_ps = psA.tile([128, 512], F32, tag="proj", bufs=2)
                    nc.tensor.matmul(hp_ps, w1_r[:, ts(j, 128)], kT_r,
                                     start=True, stop=True)
                    nc.scalar.activation(out=hact_h[:, j, :], in_=hp_ps,
                                         func=AF.Gelu)
                    nc.scalar.activation(out=dgel[:, j, :], in_=hp_ps,
                                         func=AF.Derivative_Gelu)
                hh_ps = psA.tile([128, 512], F32, tag="proj", bufs=2)
                for j in range(4):
                    nc.tensor.matmul(hh_ps, w2_h[:, ts(j, 128)], hact_h[:, j, :],
                                     start=(j == 0), stop=(j == 3))
                hhsb = pA.tile([128, 512], F32, tag="hhsb")
                nc.vector.tensor_copy(out=hhsb, in_=hh_ps)
                sq2 = pA.tile([128, 512], F16, tag="sq2", bufs=2)
                nc.scalar.activation(out=sq2, in_=hh_ps, func=AF.Square)
                ms_ps = psA.tile([1, 512], F32, tag="mix", bufs=2)
                nc.tensor.matmul(ms_ps, ones_col_h, sq2, start=True, stop=True)
                rowt2 = pA.tile([1, 512], F32, tag="rows", bufs=10)
                nc.scalar.activation(out=rowt2, in_=ms_ps, func=AF.Sqrt,
                                     scale=1.0 / DH, bias=epsT[0:1, :])
                srs_f = pA.tile([1, 512], F32, tag="rows", bufs=10)
                nc.vector.reciprocal(out=srs_f, in_=rowt2)
                srs_h = pA.tile([1, 512], F16, tag="rows", bufs=10)
                nc.scalar.copy(out=srs_h, in_=srs_f)
                srsb_ps = psA.tile([128, 512], F32, tag="bc", bufs=2)
                nc.tensor.matmul(srsb_ps, ones_row_h, srs_h, start=True, stop=True)
                ysb = pA.tile([128, 512], F32, tag="ysb")
                nc.vector.tensor_mul(out=ysb, in0=hhsb, in1=srsb_ps)
                dp = pA.tile([128, 512], F32, tag="dp")
                nc.vector.scalar_tensor_tensor(out=dp, in0=ysb, scalar=gamma,
                                               in1=kvT, op0=AX.mult, op1=AX.add)
                nc.vector.tensor_mul(out=dp, in0=dp, in1=lrB)
                gp = pA.tile([128, 512], F32, tag="gp", bufs=2)
                nc.vector.tensor_mul(out=gp, in0=dp, in1=ysb)
                nc.vector.tensor_reduce(out=gG[:, csl],
                                        in_=gp.rearrange("p (c k) -> p c k", k=CHUNK),
                                        axis=X_AXIS, op=AX.add)
                dY = pA.tile([128, 512], F32, tag="dY")
                nc.vector.tensor_scalar_mul(out=dY, in0=dp, scalar1=gamma)
                dprod = pA.tile([128, 512], F16, tag="dprod", bufs=2)
                nc.vector.tensor_mul(out=dprod, in0=dY, in1=hhsb)
                dot_ps = psA.tile([1, 512], F32, tag="mix", bufs=2)
                nc.tensor.matmul(dot_ps, ones_col_h, dprod, start=True, stop=True)
                s3 = pA.tile([1, 512], F32, tag="rows", bufs=10)
                nc.vector.tensor_mul(out=s3, in0=srs_f, in1=srs_f)
                nc.vector.tensor_mul(out=s3, in0=s3, in1=srs_f)
                c_f = pA.tile([1, 512], F32, tag="rows", bufs=10)
                nc.vector.tensor_mul(out=c_f, in0=s3, in1=dot_ps)
                c_h = pA.tile([1, 512], F16, tag="rows", bufs=10)
                nc.scalar.activation(out=c_h, in_=c_f, func=AF.Copy, scale=1.0 / DH)
                cb_ps = psA.tile([128, 512], F32, tag="bc", bufs=2)
                nc.tensor.matmul(cb_ps, ones_row_h, c_h, start=True, stop=True)
                m1t = pA.tile([128, 512], F32, tag="m1t", bufs=2)
                nc.vector.tensor_mul(out=m1t, in0=dY, in1=srsb_ps)
                m2t = pA.tile([128, 512], F32, tag="m2t", bufs=2)
                nc.vector.tensor_mul(out=m2t, in0=hhsb, in1=cb_ps)
                dhh_h = pA.tile([128, 512], F16, tag="dhh_h")
                nc.vector.tensor_sub(out=dhh_h, in0=m1t, in1=m2t)

                # backward to dhpre (fp16)
                dhpre_h = pA.tile([128, 4, 512], F16, tag="dhpre_h")
                for j in range(4):
                    da_ps = psA.tile([128, 512], F32, tag="proj", bufs=2)
                    nc.tensor.matmul(da_ps, w2T_h[:, ts(j, 128)], dhh_h,
                                     start=True, stop=True)
                    nc.vector.tensor_mul(out=dhpre_h[:, j, :], in0=da_ps,
                                         in1=dgel[:, j, :])

                # token-major transposes (fp16) -> staging -> chunk-major DRAM
                st_kc = pA.tile([128, 4, 128], F16, tag="st_kc", bufs=1)
                st_dh = pA.tile([128, 4, 128], F16, tag="st_dh", bufs=1)
                st_dp = pA.tile([128, 4, 512], F16, tag="st_dp", bufs=1)
                st_ha = pA.tile([128, 4, 512], F16, tag="st_ha", bufs=1)
                for blk in range(4):
                    bsl = ts(blk, 128)
                    tp_ps = psA.tile([128, 4, 128], F16, tag="tp", bufs=2)
                    nc.tensor.transpose(tp_ps[:, 0, :], kT_h[:, bsl], ident_h)
                    nc.tensor.transpose(tp_ps[:, 1, :], dhh_h[:, bsl], ident_h)
                    nc.vector.tensor_copy(out=st_kc[:, blk, :], in_=tp_ps[:, 0, :])
                    nc.vector.tensor_copy(out=st_dh[:, blk, :], in_=tp_ps[:, 1, :])
                    for j in range(4):
                        t2_ps = psA.tile([128, 4, 128], F16, tag="tp", bufs=2)
                        nc.tensor.transpose(t2_ps[:, 0, :], dhpre_h[:, j, bsl],
                                            ident_h)
                        nc.tensor.transpose(t2_ps[:, 1, :], hact_h[:, j, bsl],
                                            ident_h)
                        nc.vector.tensor_copy(out=st_dp[:, blk, ts(j, 128)],
                                              in_=t2_ps[:, 0, :])
                        nc.vector.tensor_copy(out=st_ha[:, blk, ts(j, 128)],
                                              in_=t2_ps[:, 1, :])
                for cm, stg in [(kc_st, st_kc), (dhh_st, st_dh),
                                (dhpre_st, st_dp), (hact_st, st_ha)]:
                    v = cm.rearrange("p (a two) x -> p a two x", two=2)
                    dma(v[:, 4 * tt:4 * tt + 4, 0, :], stg[0:64, :, :])
                    dma(v[:, 4 * tt:4 * tt + 4, 1, :], stg[64:128, :, :])

            # finish mom/dec (all chunks) -> md_st = [mom, 1-dec, gG, pad]
            mds = pA.tile([2, NCH], F32, tag="mds")
            nc.scalar.activation(out=mds, in_=mdraw, func=AF.Sigmoid,
                                 scale=1.0 / CHUNK, bias=bias_md)
            mrow_f = pA.tile([1, NCH], F32, tag="mrow_f")
            drow_f = pA.tile([1, NCH], F32, tag="drow_f")
            dma(mrow_f, mds[0:1, :])
            dma(drow_f, mds[1:2, :])
            mrow = pA.tile([1, NCH], F16, tag="mrow")
            drow = pA.tile([1, NCH], F16, tag="drow")
            nc.scalar.copy(out=mrow, in_=mrow_f)
            nc.scalar.copy(out=drow, in_=drow_f)
            mb_ps = psA.tile([128, 512], F32, tag="bc", bufs=2)
            nc.tensor.matmul(mb_ps[:, 0:NCH], ones_row_h, mrow, start=True, stop=True)
            nc.tensor.matmul(mb_ps[:, 64:64 + NCH], ones_row_h, drow,
                             start=True, stop=True)
            momB = pA.tile([128, NCH], F32, tag="momB")
            nc.vector.tensor_copy(out=momB, in_=mb_ps[:, 0:NCH])
            decm1B = pA.tile([128, NCH], F32, tag="decm1B")
            nc.scalar.activation(out=decm1B, in_=mb_ps[:, 64:64 + NCH],
                                 func=AF.Identity, scale=-1.0, bias=1.0)
            nc.vector.tensor_scalar_mul(out=gG, in0=gG, scalar1=-2.0 / DH)
            mdv = md_st.rearrange("p c x -> p x c")
            dma(mdv[:, 0, :], momB)
            dma(mdv[:, 1, :], decm1B)
            dma(mdv[:, 2, :], gG)

        # ============ CHUNK LOOP: retrieval + grads + NS5 + scans ============
        with tc.tile_pool(name="phL", bufs=1) as pL, \
             tc.tile_pool(name="psL", bufs=1, space="PSUM") as psL:
            with tc.For_i(0, NCH, 1) as c:
                kc_t = pL.tile([64, 128], F16, tag="kc_t", bufs=2)
                dma(kc_t, kc_st[:, ds(c, 1), :].rearrange("p one x -> p (one x)"))
                dhh_t = pL.tile([64, 128], F16, tag="dhh_t", bufs=2)
                dma(dhh_t, dhh_st[:, ds(c, 1), :].rearrange("p one x -> p (one x)"))
                dhpre_t = pL.tile([64, 512], F16, tag="dhpre_t", bufs=2)
                dma(dhpre_t,
                    dhpre_st[:, ds(c, 1), :].rearrange("p one x -> p (one x)"))
                hact_t = pL.tile([64, 512], F16, tag="hact_t", bufs=2)
                dma(hact_t,
                    hact_st[:, ds(c, 1), :].rearrange("p one x -> p (one x)"))
                q_t = pL.tile([128, CHUNK], F16, tag="q_t", bufs=2)
                dma(q_t, q_st[:, ds(c, 1), :].rearrange("p one x -> p (one x)"))
                gate_t = pL.tile([128, CHUNK], F16, tag="gate_t", bufs=2)
                dma(gate_t, g_st[:, ds(c, 1), :].rearrange("p one x -> p (one x)"))
                md_t = pL.tile([128, 4], F32, tag="md_t", bufs=2)
                dma(md_t, md_st[:, ds(c, 1), :].rearrange("p one x -> p (one x)"))

                # ---- retrieval with pre-update state ----
                hp_ps = psL.tile([128, 4, CHUNK], F32, tag="pr", bufs=1)
                for j in range(4):
                    nc.tensor.matmul(hp_ps[:, j, :], u1h[:, ts(j, 128)], q_t,
                                     start=True, stop=True)
                ha_c = pL.tile([128, 4, CHUNK], F16, tag="ha_c", bufs=2)
                nc.scalar.activation(out=ha_c, in_=hp_ps, func=AF.Gelu)
                hh_ps = psL.tile([128, CHUNK], F32, tag="pr", bufs=1)
                for j in range(4):
                    nc.tensor.matmul(hh_ps, u2h[:, ts(j, 128)], ha_c[:, j, :],
                                     start=(j == 0), stop=(j == 3))
                sqc = pL.tile([128, CHUNK], F16, tag="sqc", bufs=2)
                nc.scalar.activation(out=sqc, in_=hh_ps, func=AF.Square)
                hhc = pL.tile([128, CHUNK], F32, tag="hhc", bufs=2)
                nc.scalar.copy(out=hhc, in_=hh_ps)
                ms_ps = psL.tile([1, CHUNK], F32, tag="prow", bufs=1)
                nc.tensor.matmul(ms_ps, ones_col_h, sqc, start=True, stop=True)
                rr = pL.tile([1, CHUNK], F32, tag="rr", bufs=2)
                nc.scalar.activation(out=rr, in_=ms_ps, func=AF.Sqrt,
                                     scale=1.0 / DH, bias=epsT[0:1, :])
                rr2 = pL.tile([1, CHUNK], F32, tag="rr2", bufs=2)
                nc.vector.reciprocal(out=rr2, in_=rr)
                rrh = pL.tile([1, CHUNK], F16, tag="rrh", bufs=2)
                nc.scalar.copy(out=rrh, in_=rr2)
                sb_ps = psL.tile([128, CHUNK], F32, tag="pr", bufs=1)
                nc.tensor.matmul(sb_ps, ones_row_h, rrh, start=True, stop=True)
                yc = pL.tile([128, CHUNK], F32, tag="yc", bufs=2)
                nc.vector.tensor_mul(out=yc, in0=hhc, in1=sb_ps)
                prc = pL.tile([128, CHUNK], F32, tag="prc", bufs=2)
                nc.vector.scalar_tensor_tensor(out=prc, in0=yc, scalar=ugv,
                                               in1=q_t, op0=AX.mult, op1=AX.add)
                outc = pL.tile([128, CHUNK], F16, tag="outc", bufs=2)
                nc.vector.tensor_mul(out=outc, in0=prc, in1=gate_t)
                dma(d["out"].ap()[ds(c, 1)].rearrange("one p x -> (one p) x"),
                    outc)

                # ---- inner grads (g1 = dL/dw1, g2t = (dL/dw2)^T, both [dh,hid])
                g1_ps = psL.tile([128, 512], F32, tag="pg", bufs=2)
                nc.tensor.matmul(g1_ps, kc_t, dhpre_t, start=True, stop=True)
                g2_ps = psL.tile([128, 512], F32, tag="pg", bufs=2)
                nc.tensor.matmul(g2_ps, dhh_t, hact_t, start=True, stop=True)
                g1T_ps = psL.tile([128, 4, 128], F32, tag="pgT", bufs=2)
                for j in range(4):
                    nc.tensor.matmul(g1T_ps[:, j, :], dhpre_t[:, ts(j, 128)],
                                     kc_t, start=True, stop=True)
                g2T_ps = psL.tile([128, 4, 128], F32, tag="pgT", bufs=2)
                for j in range(4):
                    nc.tensor.matmul(g2T_ps[:, j, :], hact_t[:, ts(j, 128)],
                                     dhh_t, start=True, stop=True)
                g1sb = pL.tile([128, 512], F32R, tag="g1sb", bufs=2)
                nc.vector.tensor_copy(out=g1sb, in_=g1_ps)
                g2sb = pL.tile([128, 512], F32R, tag="g2sb", bufs=2)
                nc.vector.tensor_copy(out=g2sb, in_=g2_ps)
                R = pL.tile([128, 2], F32, tag="R", bufs=2)
                scr = pL.tile([128, 512], F16, tag="scr", bufs=2)
                nc.vector.scalar_tensor_tensor(
                    out=scr, in0=g1sb.bitcast(F32), scalar=1.0,
                    in1=g1sb.bitcast(F32), op0=AX.mult, op1=AX.mult,
                    accum_out=R[:, 0:1])
                scr2 = pL.tile([128, 512], F16, tag="scr", bufs=2)
                nc.vector.scalar_tensor_tensor(
                    out=scr2, in0=g2sb.bitcast(F32), scalar=1.0,
                    in1=g2sb.bitcast(F32), op0=AX.mult, op1=AX.mult,
                    accum_out=R[:, 1:2])
                Rh = pL.tile([128, 2], F16, tag="Rh", bufs=2)
                nc.vector.tensor_copy(out=Rh, in_=R)
                nrm_ps = psL.tile([1, 2], F32, tag="prow", bufs=1)
                nc.tensor.matmul(nrm_ps, ones_col_h, Rh, start=True, stop=True)
                nrm_sb = pL.tile([1, 2], F32, tag="nrm_sb", bufs=2)
                nc.vector.tensor_copy(out=nrm_sb, in_=nrm_ps)
                inv2 = pL.tile([1, 2], F32, tag="inv2", bufs=2)
                nc.vector.reciprocal_approx_fast(inv2, nrm_sb)
                ninv = pL.tile([1, 2], F32, tag="ninv", bufs=2)
                nc.scalar.activation(out=ninv, in_=inv2, func=AF.Sqrt)
                nc.scalar.activation(out=ninv, in_=ninv, func=AF.Copy, scale=-1.0)
                nb = pL.tile([128, 2], F32, tag="nb", bufs=2)
                nc.gpsimd.partition_broadcast(nb, ninv)

                # ---- NS5, fp32 (f32r matmuls), transpose-free ----
                tP = [None, None]
                tT = [None, None]
                tP[0] = pL.tile([128, 512], F32R, tag="tPa", bufs=2, name="tP0")
                nc.vector.tensor_scalar_mul(out=tP[0], in0=g1sb.bitcast(F32),
                                            scalar1=nb[:, 0:1])
                tT[0] = pL.tile([128, 4, 128], F32R, tag="tTa", bufs=2, name="tT0")
                nc.vector.tensor_scalar_mul(out=tT[0], in0=g1T_ps,
                                            scalar1=nb[:, 0:1])
                tP[1] = pL.tile([128, 512], F32R, tag="tPb", bufs=2, name="tP1")
                nc.vector.tensor_scalar_mul(out=tP[1], in0=g2sb.bitcast(F32),
                                            scalar1=nb[:, 1:2])
                tT[1] = pL.tile([128, 4, 128], F32R, tag="tTb", bufs=2, name="tT1")
                nc.vector.tensor_scalar_mul(out=tT[1], in0=g2T_ps,
                                            scalar1=nb[:, 1:2])

                for k in range(5):
                    last = k == 4
                    for i in range(2):
                        A_ps = psL.tile([128, 128], F32, tag="pA", bufs=2)
                        for j in range(4):
                            nc.tensor.matmul(A_ps, tT[i][:, j, :], tT[i][:, j, :],
                                             start=(j == 0), stop=(j == 3))
                        Ab = pL.tile([128, 128], F32R, tag="Ab", bufs=2)
                        nc.vector.tensor_scalar_mul(out=Ab, in0=A_ps, scalar1=NSB)
                        Au = pL.tile([128, 128], F32R, tag="Au", bufs=2)
                        nc.scalar.copy(out=Au, in_=A_ps)
                        A2_ps = psL.tile([128, 128], F32, tag="pA", bufs=2)
                        nc.tensor.matmul(A2_ps, Ab, Au, start=True, stop=False)
                        nc.tensor.matmul(A2_ps, identr, aIc, start=False, stop=True)
                        Bm = pL.tile([128, 128], F32R, tag="Bm", bufs=2)
                        nc.vector.scalar_tensor_tensor(
                            out=Bm, in0=A2_ps, scalar=NSC / NSB,
                            in1=Ab.bitcast(F32), op0=AX.mult, op1=AX.add)
                        if not (last and i == 1):
                            tp_ps = psL.tile([128, 512], F32, tag="pg", bufs=2)
                            nc.tensor.matmul(tp_ps, Bm, tP[i], start=True,
                                             stop=True)
                            tPn = pL.tile([128, 512], F32R,
                                          tag=("tPa" if i == 0 else "tPb"), bufs=2)
                            nc.scalar.copy(out=tPn, in_=tp_ps)
                        else:
                            tPn = tP[i]
                        if not (last and i == 0):
                            tt_ps = psL.tile([128, 4, 128], F32, tag="pgT", bufs=2)
                            for j in range(4):
                                nc.tensor.matmul(tt_ps[:, j, :],
                                                 tP[i][:, ts(j, 128)], Bm,
                                                 start=True, stop=True)
                            tTn = pL.tile([128, 4, 128], F32R,
                                          tag=("tTa" if i == 0 else "tTb"), bufs=2)
                            nc.scalar.copy(out=tTn, in_=tt_ps)
                        else:
                            tTn = tT[i]
                        tP[i] = tPn
                        tT[i] = tTn

                s1 = tP[0].bitcast(F32)
                s2 = tT[1].rearrange("p a b -> p (a b)").bitcast(F32)

                # ---- scans (momentum then weight-decay), fp32 ----
                nc.vector.scalar_tensor_tensor(out=m1s, in0=m1s,
                                               scalar=md_t[:, 0:1], in1=s1,
                                               op0=AX.mult, op1=AX.add)
                nc.vector.scalar_tensor_tensor(out=u1, in0=u1,
                                               scalar=md_t[:, 1:2], in1=m1s,
                                               op0=AX.mult, op1=AX.add)
                nc.scalar.copy(out=u1h, in_=u1)
                nc.vector.scalar_tensor_tensor(out=m2s, in0=m2s,
                                               scalar=md_t[:, 0:1], in1=s2,
                                               op0=AX.mult, op1=AX.add)
                nc.vector.scalar_tensor_tensor(out=u2, in0=u2,
                                               scalar=md_t[:, 1:2], in1=m2s,
                                               op0=AX.mult, op1=AX.add)
                nc.scalar.copy(out=u2h, in_=u2)
                nc.vector.scalar_tensor_tensor(out=mgv, in0=mgv,
                                               scalar=md_t[:, 0:1],
                                               in1=md_t[:, 2:3],
                                               op0=AX.mult, op1=AX.add)
                nc.vector.scalar_tensor_tensor(out=ugv, in0=ugv,
                                               scalar=md_t[:, 1:2], in1=mgv,
                                               op0=AX.mult, op1=AX.add)


# ------------------- host side -------------------

def _prep_core_inputs(inputs, b, h):
    f = np.float32
    sg = np.asarray(inputs["store_g"], f)[:, None]
    rg = np.asarray(inputs["retrieve_g"], f)[:, None]
    hs = slice(h * DH, (h + 1) * DH)

    def tile128(w):  # (512, X) -> rows grouped as (128, 4, X) -> (128, 4*X)
        w = np.asarray(w, f)
        return np.ascontiguousarray(
            w.reshape(4, 128, -1).transpose(1, 0, 2).reshape(128, -1))

    wk = tile128(sg * np.asarray(inputs["Wk"], f)[:, hs])
    wv = tile128(sg * np.asarray(inputs["Wv"], f)[:, hs])
    wq = tile128(rg * np.asarray(inputs["Wq"], f)[:, hs])
    wsm = tile128(np.stack([
        sg[:, 0] * np.asarray(inputs["W_lr"], f)[:, h],
        sg[:, 0] * np.asarray(inputs["Wm"], f)[:, h],
        sg[:, 0] * np.asarray(inputs["Wd"], f)[:, h],
        rg[:, 0] * np.asarray(inputs["Wgate"], f)[:, h]], axis=1))
    ident = np.eye(128, dtype=f)
    w1 = np.asarray(inputs["mw1"], f)[h]
    w2 = tile128(np.asarray(inputs["mw2"], f)[h])
    cw16 = np.concatenate([wk, wv, wq, wsm, ident, w1, w2],
                          axis=1).astype(np.float16)

    gamma = np.asarray(inputs["mgamma"], f)[h].reshape(128, 1)
    biasB = np.broadcast_to(
        np.array([inputs["b_lr"][h], 0.0, 0.0, 0.0], f), (128, 4))
    mdcol = np.zeros((128, 1), f)
    mdcol[0, 0] = inputs["bm"][h]
    mdcol[1, 0] = inputs["bd"][h]
    cw32 = np.ascontiguousarray(
        np.concatenate([gamma, biasB, mdcol], axis=1), f)

    seqT = np.asarray(inputs["seq"], f)[b].T.astype(np.float16)
    return {"seqT": np.ascontiguousarray(seqT), "cw16": cw16, "cw32": cw32}


_CACHE = {}


def _get_module():
    if "nc" not in _CACHE:
        nc = bacc.Bacc("TRN2", target_bir_lowering=False, debug=False)
        build(nc)
        nc.compile()
        _CACHE["nc"] = nc
    return _CACHE["nc"]


def kernel(**inputs):
    from concourse.bass_utils import run_bass_kernel_spmd
    nc = _get_module()
    in_maps = [_prep_core_inputs(inputs, core // HEADS, core % HEADS)
               for core in range(8)]
    res = run_bass_kernel_spmd(nc, in_maps, core_ids=list(range(8)))
    _CACHE["last_res"] = res
    Wc = np.asarray(inputs["Wc"], np.float32)
    out = np.zeros((B, N, DIM), np.float32)
    for b in range(B):
        heads = []
        for h in range(HEADS):
            oc = res.results[b * HEADS + h]["out"]  # (NCH, 128, CHUNK) f16
            heads.append(
                oc.transpose(0, 2, 1).reshape(N, DH).astype(np.float32))
        out[b] = np.concatenate(heads, axis=1) @ Wc
    return out


if __name__ == "__main__":
    dd = np.load("/root/problem/ref_inputs.npz")
    inputs = {k: dd[k] for k in dd.files}
    out = kernel(**inputs)
    exp = np.load("/root/problem/ref_expected.npy")
    err = np.abs(out - exp).max() / np.abs(exp).max()
    rel = np.linalg.norm(out - exp) / np.linalg.norm(exp)
    print(f"absmax-rel: {err:.3e}  l2-rel: {rel:.3e}")


# revision 16
# speedup vs baseline: 5.2314x; 1.3012x over previous
"""Trainium2 Bass kernel for nn_NeuralMemory (Titans-style neural memory).

Sharding: 8 cores <-> 8 (batch, head) pairs. Each core runs the full
per-(b,h) pipeline; the host applies the final Wc projection and sums
the 4 head partials per batch (268 MFLOP of BLAS, ~ms).

This revision optimizes END-TO-END dispatch cost, which dominates the
measured time on the axon/PJRT path (per-call jit+compile-cache+load
scales with program size; transfers scale with I/O bytes):
  - I/O contract shrunk ~4x: fp16 seq, packed fp16/fp32 weight tensors,
    fp16 [NCH,128,64] gated output (pre-Wc); dead inputs dropped.
  - Instruction count ~25x smaller: the 32-chunk pipeline (inner grads,
    NS5, momentum/decay scans, retrieval) runs in ONE hardware For_i
    loop instead of being fully unrolled.
  - Newton-Schulz-5 runs in fp32 (f32r matmuls) — device time is cheap
    here, and it cuts rel-err ~15x vs the old fp16 NS (1e-3 vs 1.6e-2).
    Transpose-free NS: maintain t [dh,hid] and tT blocks; with
    Bm' = aI + bA + cA^2 (A = t t^T, symmetric), t_new = Bm' t and
    tT_new_j = t_j^T Bm' are plain matmuls off the OLD t — no PE
    transposes inside the iteration.

Math restructuring (validated vs the jax reference in numpy at ~1.4e-5
fp32 / ~1.0e-3 with fp16 phase A):
  - rmsnorm gains folded into projection weights (host-side).
  - inner-loss grads derived manually at the shared initial fast
    weights; the 2/DH*lr factor is dropped for g1/g2 (NS is
    scale-invariant) and applied only to the gamma grad.
  - momentum/decay scans fused per chunk with retrieval (which uses the
    weights from the end of the previous chunk).
"""
import sys

sys.path.insert(0, "/opt/trn_rl_repo")

import numpy as np

import concourse.bass as bass
import concourse.bacc as bacc
import concourse.mybir as mybir
import concourse.tile as tile
from concourse.bass import ts, ds

F32 = mybir.dt.float32
F32R = mybir.dt.float32r
F16 = mybir.dt.float16

DIM, HEADS, DH, CHUNK = 512, 4, 128, 64
HID = DH * 4
B, N = 2, 2048
NCH = N // CHUNK          # 32 chunks
NTT = N // 512            # 4 token tiles
NSA, NSB, NSC = 3.4445, -4.775, 2.0315
AX = mybir.AluOpType
AF = mybir.ActivationFunctionType
X_AXIS = mybir.AxisListType.X

# packed fp16 const columns: wk | wv | wq | wsm | ident | w1 | w2
C16_WK, C16_WV, C16_WQ, C16_WSM, C16_ID = 0, 512, 1024, 1536, 1552
C16_W1, C16_W2 = 1680, 2192
K16 = 2192 + 512
# packed fp32 const columns: gamma | biasB | md-bias col
C32_G, C32_BB, C32_MD = 0, 1, 5
K32 = 6


def build(nc):
    d = {}
    # per-core token-quarter of seq^T; the full sequence is reassembled
    # on-device with an AllGather over each batch's 4 head-cores
    d["seqq"] = nc.dram_tensor("seqq", [DIM, N // 4], F16, kind="ExternalInput")
    d["cw16"] = nc.dram_tensor("cw16", [128, K16], F16, kind="ExternalInput")
    d["cw32"] = nc.dram_tensor("cw32", [128, K32], F32, kind="ExternalInput")
    d["out"] = nc.dram_tensor("out", [NCH, 128, CHUNK], F16, kind="ExternalOutput")
    with tile.TileContext(nc) as tc:
        _body(nc, tc, d)
    return nc


def _body(nc, tc, d):
    def dma(out, in_):
        nc.sync.dma_start(out=out, in_=in_)

    consts_cm = tc.tile_pool(name="consts", bufs=1)
    persist_cm = tc.tile_pool(name="persist", bufs=1)
    dram_cm = tc.tile_pool(name="dstage", bufs=1, space="DRAM")
    with consts_cm as consts, persist_cm as persist, dram_cm as dstage:
        # ---------------- constants ----------------
        cw16 = consts.tile([128, K16], F16)
        dma(cw16, d["cw16"].ap())
        cw32 = consts.tile([128, K32], F32)
        dma(cw32, d["cw32"].ap())
        wk_h = cw16[:, C16_WK:C16_WK + 512]
        wv_h = cw16[:, C16_WV:C16_WV + 512]
        wq_h = cw16[:, C16_WQ:C16_WQ + 512]
        wsm_h = cw16[:, C16_WSM:C16_WSM + 16]
        ident_h = cw16[:, C16_ID:C16_ID + 128]
        gamma = cw32[:, C32_G:C32_G + 1]
        biasB = cw32[:, C32_BB:C32_BB + 4]
        bias_md = cw32[0:2, C32_MD:C32_MD + 1]

        epsT = consts.tile([128, 1], F32)
        nc.vector.memset(epsT, 1e-6)
        ones_col_h = consts.tile([128, 1], F16)
        nc.vector.memset(ones_col_h, 1.0)
        ones_row_h = consts.tile([1, 128], F16)
        nc.vector.memset(ones_row_h, 1.0)
        identr = consts.tile([128, 128], F32R)
        nc.vector.tensor_copy(out=identr, in_=ident_h)
        aIc = consts.tile([128, 128], F32R)
        nc.scalar.activation(out=aIc, in_=identr.bitcast(F32), func=AF.Copy,
                             scale=NSA * NSB / NSC)
        w1_h = cw16[:, C16_W1:C16_W1 + 512]
        w2_h = cw16[:, C16_W2:C16_W2 + 512]
        w1_r = consts.tile([128, 512], F32R)
        nc.vector.tensor_copy(out=w1_r, in_=w1_h)

        # -------- persistent state --------
        u1 = persist.tile([128, 512], F32)
        u2 = persist.tile([128, 512], F32)
        u1h = persist.tile([128, 512], F16)
        u2h = persist.tile([128, 512], F16)
        m1s = persist.tile([128, 512], F32)
        m2s = persist.tile([128, 512], F32)
        ugv = persist.tile([128, 1], F32)
        mgv = persist.tile([128, 1], F32)
        mdraw = persist.tile([2, NCH], F32)
        gG = persist.tile([128, NCH], F32)
        w2T_h = persist.tile([128, 512], F16)
        nc.vector.tensor_copy(out=u1, in_=w1_h)
        nc.vector.tensor_copy(out=u2, in_=w2_h)
        nc.vector.tensor_copy(out=u1h, in_=w1_h)
        nc.vector.tensor_copy(out=u2h, in_=w2_h)
        nc.vector.tensor_copy(out=ugv, in_=gamma)
        nc.vector.memset(m1s, 0.0)
        nc.vector.memset(m2s, 0.0)
        nc.vector.memset(mgv, 0.0)

        # -------- DRAM staging (chunk-indexed) --------
        kc_st = dstage.tile([64, NCH, 128], F16)
        dhh_st = dstage.tile([64, NCH, 128], F16)
        dhpre_st = dstage.tile([64, NCH, 512], F16)
        hact_st = dstage.tile([64, NCH, 512], F16)
        q_st = dstage.tile([128, NCH, CHUNK], F16)
        g_st = dstage.tile([128, NCH, CHUNK], F16)
        md_st = dstage.tile([128, NCH, 4], F32)

        # gather the full sequence from the 4 head-cores of this batch
        seq_in = dstage.tile([DIM, N // 4], F16)
        seq_g = dstage.tile([4, DIM, N // 4], F16)
        dma(seq_in, d["seqq"].ap())
        nc.gpsimd.collective_compute(
            "AllGather", AX.bypass,
            replica_groups=[[0, 1, 2, 3], [4, 5, 6, 7]],
            ins=[seq_in.opt()], outs=[seq_g.opt()])

        # ================= PHASE A: store-side, streamed per token-tile ========
        with tc.tile_pool(name="phA", bufs=1) as pA, \
             tc.tile_pool(name="psA", bufs=1, space="PSUM") as psA:
            # w2T (dh, hid) from w2 tiles via PE transpose
            for j in range(4):
                tp_ps = psA.tile([128, 128], F16, tag="tp", bufs=2)
                nc.tensor.transpose(tp_ps, w2_h[:, ts(j, 128)], ident_h)
                nc.vector.tensor_copy(out=w2T_h[:, ts(j, 128)], in_=tp_ps)

            for tt in range(NTT):
                tsl = ts(tt, 512)
                csl = slice(tt * 8, (tt + 1) * 8)
                seq_t = pA.tile([128, 4, 512], F16, tag="seq_t", bufs=2)
                dma(seq_t, seq_g[tt].rearrange("(a p) t -> p a t", p=128))
                # rmsnorm scale
                ss_ps = psA.tile([1, 512], F32, tag="mix", bufs=2)
                for j in range(4):
                    sqs = pA.tile([128, 512], F16, tag="sqs", bufs=2)
                    nc.scalar.activation(out=sqs, in_=seq_t[:, j, :], func=AF.Square)
                    nc.tensor.matmul(ss_ps, ones_col_h, sqs,
                                     start=(j == 0), stop=(j == 3))
                rowt = pA.tile([1, 512], F32, tag="rows", bufs=10)
                nc.scalar.activation(out=rowt, in_=ss_ps, func=AF.Sqrt,
                                     scale=1.0 / DIM, bias=epsT[0:1, :])
                rs_f = pA.tile([1, 512], F32, tag="rows", bufs=10)
                nc.vector.reciprocal(out=rs_f, in_=rowt)
                rs_h = pA.tile([1, 512], F16, tag="rows", bufs=10)
                nc.scalar.copy(out=rs_h, in_=rs_f)
                rsb_ps = psA.tile([128, 512], F32, tag="bc", bufs=2)
                nc.tensor.matmul(rsb_ps, ones_row_h, rs_h, start=True, stop=True)
                sT_t = pA.tile([128, 4, 512], F16, tag="sT_t", bufs=2)
                for j in range(4):
                    nc.vector.tensor_mul(out=sT_t[:, j, :], in0=seq_t[:, j, :],
                                         in1=rsb_ps)

                # projections
                k_ps = psA.tile([128, 512], F32, tag="proj", bufs=2)
                for j in range(4):
                    nc.tensor.matmul(k_ps, wk_h[:, ts(j, 128)], sT_t[:, j, :],
                                     start=(j == 0), stop=(j == 3))
                kT_r = pA.tile([128, 512], F32R, tag="kT_r")
                nc.vector.tensor_copy(out=kT_r, in_=k_ps)
                kT_h = pA.tile([128, 512], F16, tag="kT_h")
                nc.scalar.copy(out=kT_h, in_=k_ps)
                v_ps = psA.tile([128, 512], F32, tag="proj", bufs=2)
                for j in range(4):
                    nc.tensor.matmul(v_ps, wv_h[:, ts(j, 128)], sT_t[:, j, :],
                                     start=(j == 0), stop=(j == 3))
                kvT = pA.tile([128, 512], F32, tag="kvT")
                nc.vector.tensor_sub(out=kvT, in0=kT_r.bitcast(F32), in1=v_ps)
                q_ps = psA.tile([128, 512], F32, tag="proj", bufs=2)
                for j in range(4):
                    nc.tensor.matmul(q_ps, wq_h[:, ts(j, 128)], sT_t[:, j, :],
                                     start=(j == 0), stop=(j == 3))
                q_h = pA.tile([128, 512], F16, tag="q_h", bufs=2)
                nc.scalar.copy(out=q_h, in_=q_ps)
                dma(q_st[:, csl, :], q_h.rearrange("p (c k) -> p c k", k=CHUNK))
                sm_ps = psA.tile([4, 512], F32, tag="mix", bufs=2)
                for j in range(4):
                    nc.tensor.matmul(sm_ps, wsm_h[:, ts(j, 4)], sT_t[:, j, :],
                                     start=(j == 0), stop=(j == 3))
                # copy to sbuf, then extract rows at partition 0 via tiny DMAs
                smsb = pA.tile([4, 512], F32, tag="smsb", bufs=2)
                nc.vector.tensor_copy(out=smsb, in_=sm_ps)
                lr_row = pA.tile([1, 512], F32, tag="rows", bufs=10)
                gt_row = pA.tile([1, 512], F32, tag="rows", bufs=10)
                md_rows = pA.tile([2, 512], F32, tag="md_rows", bufs=2)
                dma(lr_row, smsb[0:1, :])
                dma(gt_row, smsb[3:4, :])
                dma(md_rows, smsb[1:3, :])
                nc.vector.tensor_reduce(
                    out=mdraw[:, csl],
                    in_=md_rows.rearrange("p (c k) -> p c k", k=CHUNK),
                    axis=X_AXIS, op=AX.add)
                lr_h = pA.tile([1, 512], F16, tag="rows", bufs=10)
                nc.scalar.copy(out=lr_h, in_=lr_row)
                gt_h = pA.tile([1, 512], F16, tag="rows", bufs=10)
                nc.scalar.copy(out=gt_h, in_=gt_row)
                lg_ps = psA.tile([128, 512], F32, tag="bc", bufs=2)
                nc.tensor.matmul(lg_ps, ones_row_h, lr_h, start=True, stop=True)
                lrB = pA.tile([128, 512], F32, tag="lrB")
                nc.scalar.activation(out=lrB, in_=lg_ps, func=AF.Sigmoid,
                                     bias=biasB[:, 0:1])
                gt_ps = psA.tile([128, 512], F32, tag="bc", bufs=2)
                nc.tensor.matmul(gt_ps, ones_row_h, gt_h, start=True, stop=True)
                gate_t = pA.tile([128, 512], F16, tag="gate_t", bufs=2)
                nc.scalar.activation(out=gate_t, in_=gt_ps, func=AF.Sigmoid)
                dma(g_st[:, csl, :], gate_t.rearrange("p (c k) -> p c k", k=CHUNK))

                # forward MLP (h_pre in fp32r, rest fp16)
                hact_h = pA.tile([128, 4, 512], F16, tag="hact_h")
                dgel = pA.tile([128, 4, 512], F32, tag="dgel")
                for j in range(4):
                    hp_ps = psA.tile([128, 512], F32, tag="proj", bufs=2)
                    nc.tensor.matmul(hp_ps, w1_r[:, ts(j, 128)], kT_r,
                                     start=True, stop=True)
                    nc.scalar.activation(out=hact_h[:, j, :], in_=hp_ps,
                                         func=AF.Gelu)
                    nc.scalar.activation(out=dgel[:, j, :], in_=hp_ps,
                                         func=AF.Derivative_Gelu)
                hh_ps = psA.tile([128, 512], F32, tag="proj", bufs=2)
                for j in range(4):
                    nc.tensor.matmul(hh_ps, w2_h[:, ts(j, 128)], hact_h[:, j, :],
                                     start=(j == 0), stop=(j == 3))
                hhsb = pA.tile([128, 512], F32, tag="hhsb")
                nc.vector.tensor_copy(out=hhsb, in_=hh_ps)
                sq2 = pA.tile([128, 512], F16, tag="sq2", bufs=2)
                nc.scalar.activation(out=sq2, in_=hh_ps, func=AF.Square)
                ms_ps = psA.tile([1, 512], F32, tag="mix", bufs=2)
                nc.tensor.matmul(ms_ps, ones_col_h, sq2, start=True, stop=True)
                rowt2 = pA.tile([1, 512], F32, tag="rows", bufs=10)
                nc.scalar.activation(out=rowt2, in_=ms_ps, func=AF.Sqrt,
                                     scale=1.0 / DH, bias=epsT[0:1, :])
                srs_f = pA.tile([1, 512], F32, tag="rows", bufs=10)
                nc.vector.reciprocal(out=srs_f, in_=rowt2)
                srs_h = pA.tile([1, 512], F16, tag="rows", bufs=10)
                nc.scalar.copy(out=srs_h, in_=srs_f)
                srsb_ps = psA.tile([128, 512], F32, tag="bc", bufs=2)
                nc.tensor.matmul(srsb_ps, ones_row_h, srs_h, start=True, stop=True)
                ysb = pA.tile([128, 512], F32, tag="ysb")
                nc.vector.tensor_mul(out=ysb, in0=hhsb, in1=srsb_ps)
                dp = pA.tile([128, 512], F32, tag="dp")
                nc.vector.scalar_tensor_tensor(out=dp, in0=ysb, scalar=gamma,
                                               in1=kvT, op0=AX.mult, op1=AX.add)
                nc.vector.tensor_mul(out=dp, in0=dp, in1=lrB)
                gp = pA.tile([128, 512], F32, tag="gp", bufs=2)
                nc.vector.tensor_mul(out=gp, in0=dp, in1=ysb)
                nc.vector.tensor_reduce(out=gG[:, csl],
                                        in_=gp.rearrange("p (c k) -> p c k", k=CHUNK),
                                        axis=X_AXIS, op=AX.add)
                dY = pA.tile([128, 512], F32, tag="dY")
                nc.vector.tensor_scalar_mul(out=dY, in0=dp, scalar1=gamma)
                dprod = pA.tile([128, 512], F16, tag="dprod", bufs=2)
                nc.vector.tensor_mul(out=dprod, in0=dY, in1=hhsb)
                dot_ps = psA.tile([1, 512], F32, tag="mix", bufs=2)
                nc.tensor.matmul(dot_ps, ones_col_h, dprod, start=True, stop=True)
                s3 = pA.tile([1, 512], F32, tag="rows", bufs=10)
                nc.vector.tensor_mul(out=s3, in0=srs_f, in1=srs_f)
                nc.vector.tensor_mul(out=s3, in0=s3, in1=srs_f)
                c_f = pA.tile([1, 512], F32, tag="rows", bufs=10)
                nc.vector.tensor_mul(out=c_f, in0=s3, in1=dot_ps)
                c_h = pA.tile([1, 512], F16, tag="rows", bufs=10)
                nc.scalar.activation(out=c_h, in_=c_f, func=AF.Copy, scale=1.0 / DH)
                cb_ps = psA.tile([128, 512], F32, tag="bc", bufs=2)
                nc.tensor.matmul(cb_ps, ones_row_h, c_h, start=True, stop=True)
                m1t = pA.tile([128, 512], F32, tag="m1t", bufs=2)
                nc.vector.tensor_mul(out=m1t, in0=dY, in1=srsb_ps)
                m2t = pA.tile([128, 512], F32, tag="m2t", bufs=2)
                nc.vector.tensor_mul(out=m2t, in0=hhsb, in1=cb_ps)
                dhh_h = pA.tile([128, 512], F16, tag="dhh_h")
                nc.vector.tensor_sub(out=dhh_h, in0=m1t, in1=m2t)

                # backward to dhpre (fp16)
                dhpre_h = pA.tile([128, 4, 512], F16, tag="dhpre_h")
                for j in range(4):
                    da_ps = psA.tile([128, 512], F32, tag="proj", bufs=2)
                    nc.tensor.matmul(da_ps, w2T_h[:, ts(j, 128)], dhh_h,
                                     start=True, stop=True)
                    nc.vector.tensor_mul(out=dhpre_h[:, j, :], in0=da_ps,
                                         in1=dgel[:, j, :])

                # token-major transposes (fp16) -> staging -> chunk-major DRAM
                st_kc = pA.tile([128, 4, 128], F16, tag="st_kc", bufs=1)
                st_dh = pA.tile([128, 4, 128], F16, tag="st_dh", bufs=1)
                st_dp = pA.tile([128, 4, 512], F16, tag="st_dp", bufs=1)
                st_ha = pA.tile([128, 4, 512], F16, tag="st_ha", bufs=1)
                for blk in range(4):
                    bsl = ts(blk, 128)
                    tp_ps = psA.tile([128, 4, 128], F16, tag="tp", bufs=2)
                    nc.tensor.transpose(tp_ps[:, 0, :], kT_h[:, bsl], ident_h)
                    nc.tensor.transpose(tp_ps[:, 1, :], dhh_h[:, bsl], ident_h)
                    nc.vector.tensor_copy(out=st_kc[:, blk, :], in_=tp_ps[:, 0, :])
                    nc.vector.tensor_copy(out=st_dh[:, blk, :], in_=tp_ps[:, 1, :])
                    for j in range(4):
                        t2_ps = psA.tile([128, 4, 128], F16, tag="tp", bufs=2)
                        nc.tensor.transpose(t2_ps[:, 0, :], dhpre_h[:, j, bsl],
                                            ident_h)
                        nc.tensor.transpose(t2_ps[:, 1, :], hact_h[:, j, bsl],
                                            ident_h)
                        nc.vector.tensor_copy(out=st_dp[:, blk, ts(j, 128)],
                                              in_=t2_ps[:, 0, :])
                        nc.vector.tensor_copy(out=st_ha[:, blk, ts(j, 128)],
                                              in_=t2_ps[:, 1, :])
                for cm, stg in [(kc_st, st_kc), (dhh_st, st_dh),
                                (dhpre_st, st_dp), (hact_st, st_ha)]:
                    v = cm.rearrange("p (a two) x -> p a two x", two=2)
                    dma(v[:, 4 * tt:4 * tt + 4, 0, :], stg[0:64, :, :])
                    dma(v[:, 4 * tt:4 * tt + 4, 1, :], stg[64:128, :, :])

            # finish mom/dec (all chunks) -> md_st = [mom, 1-dec, gG, pad]
            mds = pA.tile([2, NCH], F32, tag="mds")
            nc.scalar.activation(out=mds, in_=mdraw, func=AF.Sigmoid,
                                 scale=1.0 / CHUNK, bias=bias_md)
            mrow_f = pA.tile([1, NCH], F32, tag="mrow_f")
            drow_f = pA.tile([1, NCH], F32, tag="drow_f")
            dma(mrow_f, mds[0:1, :])
            dma(drow_f, mds[1:2, :])
            mrow = pA.tile([1, NCH], F16, tag="mrow")
            drow = pA.tile([1, NCH], F16, tag="drow")
            nc.scalar.copy(out=mrow, in_=mrow_f)
            nc.scalar.copy(out=drow, in_=drow_f)
            mb_ps = psA.tile([128, 512], F32, tag="bc", bufs=2)
            nc.tensor.matmul(mb_ps[:, 0:NCH], ones_row_h, mrow, start=True, stop=True)
            nc.tensor.matmul(mb_ps[:, 64:64 + NCH], ones_row_h, drow,
                             start=True, stop=True)
            momB = pA.tile([128, NCH], F32, tag="momB")
            nc.vector.tensor_copy(out=momB, in_=mb_ps[:, 0:NCH])
            decm1B = pA.tile([128, NCH], F32, tag="decm1B")
            nc.scalar.activation(out=decm1B, in_=mb_ps[:, 64:64 + NCH],
                                 func=AF.Identity, scale=-1.0, bias=1.0)
            nc.vector.tensor_scalar_mul(out=gG, in0=gG, scalar1=-2.0 / DH)
            mdv = md_st.rearrange("p c x -> p x c")
            dma(mdv[:, 0, :], momB)
            dma(mdv[:, 1, :], decm1B)
            dma(mdv[:, 2, :], gG)

        # ============ CHUNK LOOP: retrieval + grads + NS5 + scans ============
        with tc.tile_pool(name="phL", bufs=1) as pL, \
             tc.tile_pool(name="psL", bufs=1, space="PSUM") as psL:
            with tc.For_i(0, NCH, 1) as c:
                kc_t = pL.tile([64, 128], F16, tag="kc_t", bufs=2)
                dma(kc_t, kc_st[:, ds(c, 1), :].rearrange("p one x -> p (one x)"))
                dhh_t = pL.tile([64, 128], F16, tag="dhh_t", bufs=2)
                dma(dhh_t, dhh_st[:, ds(c, 1), :].rearrange("p one x -> p (one x)"))
                dhpre_t = pL.tile([64, 512], F16, tag="dhpre_t", bufs=2)
                dma(dhpre_t,
                    dhpre_st[:, ds(c, 1), :].rearrange("p one x -> p (one x)"))
                hact_t = pL.tile([64, 512], F16, tag="hact_t", bufs=2)
                dma(hact_t,
                    hact_st[:, ds(c, 1), :].rearrange("p one x -> p (one x)"))
                q_t = pL.tile([128, CHUNK], F16, tag="q_t", bufs=2)
                dma(q_t, q_st[:, ds(c, 1), :].rearrange("p one x -> p (one x)"))
                gate_t = pL.tile([128, CHUNK], F16, tag="gate_t", bufs=2)
                dma(gate_t, g_st[:, ds(c, 1), :].rearrange("p one x -> p (one x)"))
                md_t = pL.tile([128, 4], F32, tag="md_t", bufs=2)
                dma(md_t, md_st[:, ds(c, 1), :].rearrange("p one x -> p (one x)"))

                # ---- retrieval with pre-update state ----
                hp_ps = psL.tile([128, 4, CHUNK], F32, tag="pr", bufs=1)
                for j in range(4):
                    nc.tensor.matmul(hp_ps[:, j, :], u1h[:, ts(j, 128)], q_t,
                                     start=True, stop=True)
                ha_c = pL.tile([128, 4, CHUNK], F16, tag="ha_c", bufs=2)
                nc.scalar.activation(out=ha_c, in_=hp_ps, func=AF.Gelu)
                hh_ps = psL.tile([128, CHUNK], F32, tag="pr", bufs=1)
                for j in range(4):
                    nc.tensor.matmul(hh_ps, u2h[:, ts(j, 128)], ha_c[:, j, :],
                                     start=(j == 0), stop=(j == 3))
                sqc = pL.tile([128, CHUNK], F16, tag="sqc", bufs=2)
                nc.scalar.activation(out=sqc, in_=hh_ps, func=AF.Square)
                hhc = pL.tile([128, CHUNK], F32, tag="hhc", bufs=2)
                nc.scalar.copy(out=hhc, in_=hh_ps)
                ms_ps = psL.tile([1, CHUNK], F32, tag="prow", bufs=1)
                nc.tensor.matmul(ms_ps, ones_col_h, sqc, start=True, stop=True)
                rr = pL.tile([1, CHUNK], F32, tag="rr", bufs=2)
                nc.scalar.activation(out=rr, in_=ms_ps, func=AF.Sqrt,
                                     scale=1.0 / DH, bias=epsT[0:1, :])
                rr2 = pL.tile([1, CHUNK], F32, tag="rr2", bufs=2)
                nc.vector.reciprocal(out=rr2, in_=rr)
                rrh = pL.tile([1, CHUNK], F16, tag="rrh", bufs=2)
                nc.scalar.copy(out=rrh, in_=rr2)
                sb_ps = psL.tile([128, CHUNK], F32, tag="pr", bufs=1)
                nc.tensor.matmul(sb_ps, ones_row_h, rrh, start=True, stop=True)
                yc = pL.tile([128, CHUNK], F32, tag="yc", bufs=2)
                nc.vector.tensor_mul(out=yc, in0=hhc, in1=sb_ps)
                prc = pL.tile([128, CHUNK], F32, tag="prc", bufs=2)
                nc.vector.scalar_tensor_tensor(out=prc, in0=yc, scalar=ugv,
                                               in1=q_t, op0=AX.mult, op1=AX.add)
                outc = pL.tile([128, CHUNK], F16, tag="outc", bufs=2)
                nc.vector.tensor_mul(out=outc, in0=prc, in1=gate_t)
                dma(d["out"].ap()[ds(c, 1)].rearrange("one p x -> (one p) x"),
                    outc)

                # ---- inner grads (g1 = dL/dw1, g2t = (dL/dw2)^T, both [dh,hid])
                g1_ps = psL.tile([128, 512], F32, tag="pg", bufs=2)
                nc.tensor.matmul(g1_ps, kc_t, dhpre_t, start=True, stop=True)
                g2_ps = psL.tile([128, 512], F32, tag="pg", bufs=2)
                nc.tensor.matmul(g2_ps, dhh_t, hact_t, start=True, stop=True)
                g1T_ps = psL.tile([128, 4, 128], F32, tag="pgT", bufs=2)
                for j in range(4):
                    nc.tensor.matmul(g1T_ps[:, j, :], dhpre_t[:, ts(j, 128)],
                                     kc_t, start=True, stop=True)
                g2T_ps = psL.tile([128, 4, 128], F32, tag="pgT", bufs=2)
                for j in range(4):
                    nc.tensor.matmul(g2T_ps[:, j, :], hact_t[:, ts(j, 128)],
                                     dhh_t, start=True, stop=True)
                g1sb = pL.tile([128, 512], F32R, tag="g1sb", bufs=2)
                nc.vector.tensor_copy(out=g1sb, in_=g1_ps)
                g2sb = pL.tile([128, 512], F32R, tag="g2sb", bufs=2)
                nc.vector.tensor_copy(out=g2sb, in_=g2_ps)
                R = pL.tile([128, 2], F32, tag="R", bufs=2)
                scr = pL.tile([128, 512], F16, tag="scr", bufs=2)
                nc.vector.scalar_tensor_tensor(
                    out=scr, in0=g1sb.bitcast(F32), scalar=1.0,
                    in1=g1sb.bitcast(F32), op0=AX.mult, op1=AX.mult,
                    accum_out=R[:, 0:1])
                scr2 = pL.tile([128, 512], F16, tag="scr", bufs=2)
                nc.vector.scalar_tensor_tensor(
                    out=scr2, in0=g2sb.bitcast(F32), scalar=1.0,
                    in1=g2sb.bitcast(F32), op0=AX.mult, op1=AX.mult,
                    accum_out=R[:, 1:2])
                Rh = pL.tile([128, 2], F16, tag="Rh", bufs=2)
                nc.vector.tensor_copy(out=Rh, in_=R)
                nrm_ps = psL.tile([1, 2], F32, tag="prow", bufs=1)
                nc.tensor.matmul(nrm_ps, ones_col_h, Rh, start=True, stop=True)
                nrm_sb = pL.tile([1, 2], F32, tag="nrm_sb", bufs=2)
                nc.vector.tensor_copy(out=nrm_sb, in_=nrm_ps)
                inv2 = pL.tile([1, 2], F32, tag="inv2", bufs=2)
                nc.vector.reciprocal_approx_fast(inv2, nrm_sb)
                ninv = pL.tile([1, 2], F32, tag="ninv", bufs=2)
                nc.scalar.activation(out=ninv, in_=inv2, func=AF.Sqrt)
                nc.scalar.activation(out=ninv, in_=ninv, func=AF.Copy, scale=-1.0)
                nb = pL.tile([128, 2], F32, tag="nb", bufs=2)
                nc.gpsimd.partition_broadcast(nb, ninv)

                # ---- NS5, fp32 (f32r matmuls), transpose-free ----
                tP = [None, None]
                tT = [None, None]
                tP[0] = pL.tile([128, 512], F32R, tag="tPa", bufs=2, name="tP0")
                nc.vector.tensor_scalar_mul(out=tP[0], in0=g1sb.bitcast(F32),
                                            scalar1=nb[:, 0:1])
                tT[0] = pL.tile([128, 4, 128], F32R, tag="tTa", bufs=2, name="tT0")
                nc.vector.tensor_scalar_mul(out=tT[0], in0=g1T_ps,
                                            scalar1=nb[:, 0:1])
                tP[1] = pL.tile([128, 512], F32R, tag="tPb", bufs=2, name="tP1")
                nc.vector.tensor_scalar_mul(out=tP[1], in0=g2sb.bitcast(F32),
                                            scalar1=nb[:, 1:2])
                tT[1] = pL.tile([128, 4, 128], F32R, tag="tTb", bufs=2, name="tT1")
                nc.vector.tensor_scalar_mul(out=tT[1], in0=g2T_ps,
                                            scalar1=nb[:, 1:2])

                for k in range(5):
                    last = k == 4
                    for i in range(2):
                        A_ps = psL.tile([128, 128], F32, tag="pA", bufs=2)
                        for j in range(4):
                            nc.tensor.matmul(A_ps, tT[i][:, j, :], tT[i][:, j, :],
                                             start=(j == 0), stop=(j == 3))
                        Ab = pL.tile([128, 128], F32R, tag="Ab", bufs=2)
                        nc.vector.tensor_scalar_mul(out=Ab, in0=A_ps, scalar1=NSB)
                        Au = pL.tile([128, 128], F32R, tag="Au", bufs=2)
                        nc.scalar.copy(out=Au, in_=A_ps)
                        A2_ps = psL.tile([128, 128], F32, tag="pA", bufs=2)
                        nc.tensor.matmul(A2_ps, Ab, Au, start=True, stop=False)
                        nc.tensor.matmul(A2_ps, identr, aIc, start=False, stop=True)
                        Bm = pL.tile([128, 128], F32R, tag="Bm", bufs=2)
                        nc.vector.scalar_tensor_tensor(
                            out=Bm, in0=A2_ps, scalar=NSC / NSB,
                            in1=Ab.bitcast(F32), op0=AX.mult, op1=AX.add)
                        if not (last and i == 1):
                            tp_ps = psL.tile([128, 512], F32, tag="pg", bufs=2)
                            nc.tensor.matmul(tp_ps, Bm, tP[i], start=True,
                                             stop=True)
                            tPn = pL.tile([128, 512], F32R,
                                          tag=("tPa" if i == 0 else "tPb"), bufs=2)
                            nc.scalar.copy(out=tPn, in_=tp_ps)
                        else:
                            tPn = tP[i]
                        if not (last and i == 0):
                            tt_ps = psL.tile([128, 4, 128], F32, tag="pgT", bufs=2)
                            for j in range(4):
                                nc.tensor.matmul(tt_ps[:, j, :],
                                                 tP[i][:, ts(j, 128)], Bm,
                                                 start=True, stop=True)
                            tTn = pL.tile([128, 4, 128], F32R,
                                          tag=("tTa" if i == 0 else "tTb"), bufs=2)
                            nc.scalar.copy(out=tTn, in_=tt_ps)
                        else:
                            tTn = tT[i]
                        tP[i] = tPn
                        tT[i] = tTn

                s1 = tP[0].bitcast(F32)
                s2 = tT[1].rearrange("p a b -> p (a b)").bitcast(F32)

                # ---- scans (momentum then weight-decay), fp32 ----
                nc.vector.scalar_tensor_tensor(out=m1s, in0=m1s,
                                               scalar=md_t[:, 0:1], in1=s1,
                                               op0=AX.mult, op1=AX.add)
                nc.vector.scalar_tensor_tensor(out=u1, in0=u1,
                                               scalar=md_t[:, 1:2], in1=m1s,
                                               op0=AX.mult, op1=AX.add)
                nc.scalar.copy(out=u1h, in_=u1)
                nc.vector.scalar_tensor_tensor(out=m2s, in0=m2s,
                                               scalar=md_t[:, 0:1], in1=s2,
                                               op0=AX.mult, op1=AX.add)
                nc.vector.scalar_tensor_tensor(out=u2, in0=u2,
                                               scalar=md_t[:, 1:2], in1=m2s,
                                               op0=AX.mult, op1=AX.add)
                nc.scalar.copy(out=u2h, in_=u2)
                nc.vector.scalar_tensor_tensor(out=mgv, in0=mgv,
                                               scalar=md_t[:, 0:1],
                                               in1=md_t[:, 2:3],
                                               op0=AX.mult, op1=AX.add)
                nc.vector.scalar_tensor_tensor(out=ugv, in0=ugv,
                                               scalar=md_t[:, 1:2], in1=mgv,
                                               op0=AX.mult, op1=AX.add)


# ------------------- host side -------------------

def _prep_core_inputs(inputs, b, h):
    f = np.float32
    sg = np.asarray(inputs["store_g"], f)[:, None]
    rg = np.asarray(inputs["retrieve_g"], f)[:, None]
    hs = slice(h * DH, (h + 1) * DH)

    def tile128(w):  # (512, X) -> rows grouped as (128, 4, X) -> (128, 4*X)
        w = np.asarray(w, f)
        return np.ascontiguousarray(
            w.reshape(4, 128, -1).transpose(1, 0, 2).reshape(128, -1))

    wk = tile128(sg * np.asarray(inputs["Wk"], f)[:, hs])
    wv = tile128(sg * np.asarray(inputs["Wv"], f)[:, hs])
    wq = tile128(rg * np.asarray(inputs["Wq"], f)[:, hs])
    wsm = tile128(np.stack([
        sg[:, 0] * np.asarray(inputs["W_lr"], f)[:, h],
        sg[:, 0] * np.asarray(inputs["Wm"], f)[:, h],
        sg[:, 0] * np.asarray(inputs["Wd"], f)[:, h],
        rg[:, 0] * np.asarray(inputs["Wgate"], f)[:, h]], axis=1))
    ident = np.eye(128, dtype=f)
    w1 = np.asarray(inputs["mw1"], f)[h]
    w2 = tile128(np.asarray(inputs["mw2"], f)[h])
    cw16 = np.concatenate([wk, wv, wq, wsm, ident, w1, w2],
                          axis=1).astype(np.float16)

    gamma = np.asarray(inputs["mgamma"], f)[h].reshape(128, 1)
    biasB = np.broadcast_to(
        np.array([inputs["b_lr"][h], 0.0, 0.0, 0.0], f), (128, 4))
    mdcol = np.zeros((128, 1), f)
    mdcol[0, 0] = inputs["bm"][h]
    mdcol[1, 0] = inputs["bd"][h]
    cw32 = np.ascontiguousarray(
        np.concatenate([gamma, biasB, mdcol], axis=1), f)

    seqq = np.asarray(inputs["seq"], f)[b, h * (N // 4):(h + 1) * (N // 4), :] \
        .T.astype(np.float16)
    return {"seqq": np.ascontiguousarray(seqq), "cw16": cw16, "cw32": cw32}


_CACHE = {}


def _get_module():
    if "nc" not in _CACHE:
        nc = bacc.Bacc("TRN2", target_bir_lowering=False, debug=False,
                       num_devices=8)
        build(nc)
        nc.compile()
        _CACHE["nc"] = nc
    return _CACHE["nc"]


def kernel(**inputs):
    from concourse.bass_utils import run_bass_kernel_spmd
    nc = _get_module()
    in_maps = [_prep_core_inputs(inputs, core // HEADS, core % HEADS)
               for core in range(8)]
    res = run_bass_kernel_spmd(nc, in_maps, core_ids=list(range(8)))
    _CACHE["last_res"] = res
    Wc = np.asarray(inputs["Wc"], np.float32)
    out = np.zeros((B, N, DIM), np.float32)
    for b in range(B):
        heads = []
        for h in range(HEADS):
            oc = res.results[b * HEADS + h]["out"]  # (NCH, 128, CHUNK) f16
            heads.append(
                oc.transpose(0, 2, 1).reshape(N, DH).astype(np.float32))
        out[b] = np.concatenate(heads, axis=1) @ Wc
    return out


if __name__ == "__main__":
    dd = np.load("/root/problem/ref_inputs.npz")
    inputs = {k: dd[k] for k in dd.files}
    out = kernel(**inputs)
    exp = np.load("/root/problem/ref_expected.npy")
    err = np.abs(out - exp).max() / np.abs(exp).max()
    rel = np.linalg.norm(out - exp) / np.linalg.norm(exp)
    print(f"absmax-rel: {err:.3e}  l2-rel: {rel:.3e}")


# revision 25
# speedup vs baseline: 5.2845x; 1.0101x over previous
"""Trainium2 Bass kernel for nn_NeuralMemory (Titans-style neural memory).

Sharding: 8 cores <-> 8 (batch, head) pairs. Each core runs the full
per-(b,h) pipeline; the host applies the final Wc projection and sums
the 4 head partials per batch (268 MFLOP of BLAS, ~ms).

This revision optimizes END-TO-END dispatch cost, which dominates the
measured time on the axon/PJRT path (per-call jit+compile-cache+load
scales with program size; transfers scale with I/O bytes):
  - I/O contract shrunk ~4x: fp16 seq, packed fp16/fp32 weight tensors,
    fp16 [NCH,128,64] gated output (pre-Wc); dead inputs dropped.
  - Instruction count ~25x smaller: the 32-chunk pipeline (inner grads,
    NS5, momentum/decay scans, retrieval) runs in ONE hardware For_i
    loop instead of being fully unrolled.
  - Newton-Schulz-5 runs in fp32 (f32r matmuls) — device time is cheap
    here, and it cuts rel-err ~15x vs the old fp16 NS (1e-3 vs 1.6e-2).
    Transpose-free NS: maintain t [dh,hid] and tT blocks; with
    Bm' = aI + bA + cA^2 (A = t t^T, symmetric), t_new = Bm' t and
    tT_new_j = t_j^T Bm' are plain matmuls off the OLD t — no PE
    transposes inside the iteration.

Math restructuring (validated vs the jax reference in numpy at ~1.4e-5
fp32 / ~1.0e-3 with fp16 phase A):
  - rmsnorm gains folded into projection weights (host-side).
  - inner-loss grads derived manually at the shared initial fast
    weights; the 2/DH*lr factor is dropped for g1/g2 (NS is
    scale-invariant) and applied only to the gamma grad.
  - momentum/decay scans fused per chunk with retrieval (which uses the
    weights from the end of the previous chunk).
"""
import sys

sys.path.insert(0, "/opt/trn_rl_repo")

import numpy as np

import concourse.bass as bass
import concourse.bacc as bacc
import concourse.mybir as mybir
import concourse.tile as tile
from concourse.bass import ts, ds

F32 = mybir.dt.float32
F32R = mybir.dt.float32r
F16 = mybir.dt.float16

DIM, HEADS, DH, CHUNK = 512, 4, 128, 64
HID = DH * 4
B, N = 2, 2048
NCH = N // CHUNK          # 32 chunks
NTT = N // 512            # 4 token tiles
NSA, NSB, NSC = 3.4445, -4.775, 2.0315
AX = mybir.AluOpType
AF = mybir.ActivationFunctionType
X_AXIS = mybir.AxisListType.X

# packed fp16 const columns: wk | wv | wq | wsm | ident | w1 | w2
C16_WK, C16_WV, C16_WQ, C16_WSM, C16_ID = 0, 512, 1024, 1536, 1552
C16_W1, C16_W2 = 1680, 2192
K16 = 2192 + 512
# packed fp32 const columns: gamma | biasB | md-bias col
C32_G, C32_BB, C32_MD = 0, 1, 5
K32 = 6


def build(nc):
    d = {}
    # per-core token-quarter of seq^T; the full sequence is reassembled
    # on-device with an AllGather over each batch's 4 head-cores
    d["seqq"] = nc.dram_tensor("seqq", [DIM, N // 4], F16, kind="ExternalInput")
    d["cw16"] = nc.dram_tensor("cw16", [128, K16], F16, kind="ExternalInput")
    d["cw32"] = nc.dram_tensor("cw32", [128, K32], F32, kind="ExternalInput")
    d["out"] = nc.dram_tensor("out", [NCH, 128, CHUNK], F16, kind="ExternalOutput")
    with tile.TileContext(nc) as tc:
        _body(nc, tc, d)
    return nc


def _body(nc, tc, d):
    def dma(out, in_):
        nc.sync.dma_start(out=out, in_=in_)

    consts_cm = tc.tile_pool(name="consts", bufs=1)
    persist_cm = tc.tile_pool(name="persist", bufs=1)
    dram_cm = tc.tile_pool(name="dstage", bufs=1, space="DRAM")
    with consts_cm as consts, persist_cm as persist, dram_cm as dstage:
        # ---------------- constants ----------------
        cw16 = consts.tile([128, K16], F16)
        dma(cw16, d["cw16"].ap())
        cw32 = consts.tile([128, K32], F32)
        dma(cw32, d["cw32"].ap())
        wk_h = cw16[:, C16_WK:C16_WK + 512]
        wv_h = cw16[:, C16_WV:C16_WV + 512]
        wq_h = cw16[:, C16_WQ:C16_WQ + 512]
        wsm_h = cw16[:, C16_WSM:C16_WSM + 16]
        ident_h = cw16[:, C16_ID:C16_ID + 128]
        gamma = cw32[:, C32_G:C32_G + 1]
        biasB = cw32[:, C32_BB:C32_BB + 4]
        bias_md = cw32[0:2, C32_MD:C32_MD + 1]

        epsT = consts.tile([128, 1], F32)
        nc.vector.memset(epsT, 1e-6)
        ones_col_h = consts.tile([128, 1], F16)
        nc.vector.memset(ones_col_h, 1.0)
        ones_row_h = consts.tile([1, 128], F16)
        nc.vector.memset(ones_row_h, 1.0)
        identr = consts.tile([128, 128], F32R)
        nc.vector.tensor_copy(out=identr, in_=ident_h)
        aIc = consts.tile([128, 128], F32R)
        nc.scalar.activation(out=aIc, in_=identr.bitcast(F32), func=AF.Copy,
                             scale=NSA * NSB / NSC)
        w1_h = cw16[:, C16_W1:C16_W1 + 512]
        w2_h = cw16[:, C16_W2:C16_W2 + 512]
        w1_r = consts.tile([128, 512], F32R)
        nc.vector.tensor_copy(out=w1_r, in_=w1_h)

        # -------- persistent state --------
        u1 = persist.tile([128, 512], F32)
        u2 = persist.tile([128, 512], F32)
        u1h = persist.tile([128, 512], F16)
        u2h = persist.tile([128, 512], F16)
        m1s = persist.tile([128, 512], F32)
        m2s = persist.tile([128, 512], F32)
        ugv = persist.tile([128, 1], F32)
        mgv = persist.tile([128, 1], F32)
        w2T_h = persist.tile([128, 512], F16)
        nc.vector.tensor_copy(out=u1, in_=w1_h)
        nc.vector.tensor_copy(out=u2, in_=w2_h)
        nc.vector.tensor_copy(out=u1h, in_=w1_h)
        nc.vector.tensor_copy(out=u2h, in_=w2_h)
        nc.vector.tensor_copy(out=ugv, in_=gamma)
        nc.vector.memset(m1s, 0.0)
        nc.vector.memset(m2s, 0.0)
        nc.vector.memset(mgv, 0.0)

        # -------- DRAM staging (chunk-indexed) --------
        kc_st = dstage.tile([64, NCH, 128], F16)
        dhh_st = dstage.tile([64, NCH, 128], F16)
        dhpre_st = dstage.tile([64, NCH, 512], F16)
        hact_st = dstage.tile([64, NCH, 512], F16)
        q_st = dstage.tile([128, NCH, CHUNK], F16)
        g_st = dstage.tile([128, NCH, CHUNK], F16)
        md_st = dstage.tile([128, NCH, 4], F32)

        # gather the full sequence from the 4 head-cores of this batch
        seq_in = dstage.tile([DIM, N // 4], F16)
        seq_g = dstage.tile([4, DIM, N // 4], F16)
        dma(seq_in, d["seqq"].ap())
        nc.gpsimd.collective_compute(
            "AllGather", AX.bypass,
            replica_groups=[[0, 1, 2, 3], [4, 5, 6, 7]],
            ins=[seq_in.opt()], outs=[seq_g.opt()])

        # ================= PHASE A: store-side, streamed per token-tile ========
        with tc.tile_pool(name="phA", bufs=1) as pA, \
             tc.tile_pool(name="psA", bufs=1, space="PSUM") as psA:
            # w2T (dh, hid) from w2 tiles via PE transpose
            for j in range(4):
                tp_ps = psA.tile([128, 128], F16, tag="tp", bufs=2)
                nc.tensor.transpose(tp_ps, w2_h[:, ts(j, 128)], ident_h)
                nc.vector.tensor_copy(out=w2T_h[:, ts(j, 128)], in_=tp_ps)

            with tc.For_i(0, NTT, 1) as tt:
                a8 = tt * 8
                seq_t = pA.tile([128, 4, 512], F16, tag="seq_t", bufs=2)
                dma(seq_t, seq_g[ds(tt, 1)]
                    .rearrange("one (g p) t -> p (one g) t", p=128))
                # rmsnorm scale
                ss_ps = psA.tile([1, 512], F32, tag="mix", bufs=2)
                for j in range(4):
                    sqs = pA.tile([128, 512], F16, tag="sqs", bufs=2)
                    nc.scalar.activation(out=sqs, in_=seq_t[:, j, :], func=AF.Square)
                    nc.tensor.matmul(ss_ps, ones_col_h, sqs,
                                     start=(j == 0), stop=(j == 3))
                rowt = pA.tile([1, 512], F32, tag="rows", bufs=16)
                nc.scalar.activation(out=rowt, in_=ss_ps, func=AF.Sqrt,
                                     scale=1.0 / DIM, bias=epsT[0:1, :])
                rs_f = pA.tile([1, 512], F32, tag="rows", bufs=16)
                nc.vector.reciprocal(out=rs_f, in_=rowt)
                rs_h = pA.tile([1, 512], F16, tag="rows", bufs=16)
                nc.scalar.copy(out=rs_h, in_=rs_f)
                rsb_ps = psA.tile([128, 512], F32, tag="bc", bufs=2)
                nc.tensor.matmul(rsb_ps, ones_row_h, rs_h, start=True, stop=True)
                sT_t = pA.tile([128, 4, 512], F16, tag="sT_t", bufs=2)
                for j in range(4):
                    nc.vector.tensor_mul(out=sT_t[:, j, :], in0=seq_t[:, j, :],
                                         in1=rsb_ps)

                # projections
                k_ps = psA.tile([128, 512], F32, tag="proj", bufs=2)
                for j in range(4):
                    nc.tensor.matmul(k_ps, wk_h[:, ts(j, 128)], sT_t[:, j, :],
                                     start=(j == 0), stop=(j == 3))
                kT_r = pA.tile([128, 512], F32R, tag="kT_r")
                nc.vector.tensor_copy(out=kT_r, in_=k_ps)
                kT_h = pA.tile([128, 512], F16, tag="kT_h")
                nc.scalar.copy(out=kT_h, in_=k_ps)
                v_ps = psA.tile([128, 512], F32, tag="proj", bufs=2)
                for j in range(4):
                    nc.tensor.matmul(v_ps, wv_h[:, ts(j, 128)], sT_t[:, j, :],
                                     start=(j == 0), stop=(j == 3))
                kvT = pA.tile([128, 512], F32, tag="kvT")
                nc.vector.tensor_sub(out=kvT, in0=kT_r.bitcast(F32), in1=v_ps)
                q_ps = psA.tile([128, 512], F32, tag="proj", bufs=2)
                for j in range(4):
                    nc.tensor.matmul(q_ps, wq_h[:, ts(j, 128)], sT_t[:, j, :],
                                     start=(j == 0), stop=(j == 3))
                q_h = pA.tile([128, 512], F16, tag="q_h", bufs=2)
                nc.scalar.copy(out=q_h, in_=q_ps)
                dma(q_st[:, ds(a8, 8), :],
                    q_h.rearrange("p (c k) -> p c k", k=CHUNK))
                sm_ps = psA.tile([4, 512], F32, tag="mix", bufs=2)
                for j in range(4):
                    nc.tensor.matmul(sm_ps, wsm_h[:, ts(j, 4)], sT_t[:, j, :],
                                     start=(j == 0), stop=(j == 3))
                # copy to sbuf, then extract rows at partition 0 via tiny DMAs
                smsb = pA.tile([4, 512], F32, tag="smsb", bufs=2)
                nc.vector.tensor_copy(out=smsb, in_=sm_ps)
                lr_row = pA.tile([1, 512], F32, tag="rows", bufs=16)
                gt_row = pA.tile([1, 512], F32, tag="rows", bufs=16)
                md_rows = pA.tile([2, 512], F32, tag="md_rows", bufs=2)
                dma(lr_row, smsb[0:1, :])
                dma(gt_row, smsb[3:4, :])
                dma(md_rows, smsb[1:3, :])
                # per-chunk mom/dec: sums -> sigmoid -> broadcast -> md_st
                md8 = pA.tile([2, 8], F32, tag="md8", bufs=2)
                nc.vector.tensor_reduce(
                    out=md8,
                    in_=md_rows.rearrange("p (c k) -> p c k", k=CHUNK),
                    axis=X_AXIS, op=AX.add)
                mds8 = pA.tile([2, 8], F32, tag="mds8", bufs=2)
                nc.scalar.activation(out=mds8, in_=md8, func=AF.Sigmoid,
                                     scale=1.0 / CHUNK, bias=bias_md)
                mrow8f = pA.tile([1, 8], F32, tag="rows", bufs=16)
                drow8f = pA.tile([1, 8], F32, tag="rows", bufs=16)
                dma(mrow8f, mds8[0:1, :])
                dma(drow8f, mds8[1:2, :])
                mrow8 = pA.tile([1, 8], F16, tag="rows", bufs=16)
                nc.scalar.copy(out=mrow8, in_=mrow8f)
                drow8 = pA.tile([1, 8], F16, tag="rows", bufs=16)
                nc.scalar.copy(out=drow8, in_=drow8f)
                mb8_ps = psA.tile([128, 16], F32, tag="mix", bufs=2)
                nc.tensor.matmul(mb8_ps[:, 0:8], ones_row_h, mrow8,
                                 start=True, stop=True)
                nc.tensor.matmul(mb8_ps[:, 8:16], ones_row_h, drow8,
                                 start=True, stop=True)
                momB8 = pA.tile([128, 8], F32, tag="momB8", bufs=2)
                nc.vector.tensor_copy(out=momB8, in_=mb8_ps[:, 0:8])
                decm1B8 = pA.tile([128, 8], F32, tag="decm1B8", bufs=2)
                nc.scalar.activation(out=decm1B8, in_=mb8_ps[:, 8:16],
                                     func=AF.Identity, scale=-1.0, bias=1.0)
                dma(md_st[:, ds(a8, 8), 0:1].rearrange("p c x -> p (c x)"),
                    momB8)
                dma(md_st[:, ds(a8, 8), 1:2].rearrange("p c x -> p (c x)"),
                    decm1B8)
                lr_h = pA.tile([1, 512], F16, tag="rows", bufs=16)
                nc.scalar.copy(out=lr_h, in_=lr_row)
                gt_h = pA.tile([1, 512], F16, tag="rows", bufs=16)
                nc.scalar.copy(out=gt_h, in_=gt_row)
                lg_ps = psA.tile([128, 512], F32, tag="bc", bufs=2)
                nc.tensor.matmul(lg_ps, ones_row_h, lr_h, start=True, stop=True)
                lrB = pA.tile([128, 512], F32, tag="lrB")
                nc.scalar.activation(out=lrB, in_=lg_ps, func=AF.Sigmoid,
                                     bias=biasB[:, 0:1])
                gt_ps = psA.tile([128, 512], F32, tag="bc", bufs=2)
                nc.tensor.matmul(gt_ps, ones_row_h, gt_h, start=True, stop=True)
                gate_t = pA.tile([128, 512], F16, tag="gate_t", bufs=2)
                nc.scalar.activation(out=gate_t, in_=gt_ps, func=AF.Sigmoid)
                dma(g_st[:, ds(a8, 8), :],
                    gate_t.rearrange("p (c k) -> p c k", k=CHUNK))

                # forward MLP (h_pre in fp32r, rest fp16)
                hact_h = pA.tile([128, 4, 512], F16, tag="hact_h")
                dgel = pA.tile([128, 4, 512], F32, tag="dgel")
                for j in range(4):
                    hp_ps = psA.tile([128, 512], F32, tag="proj", bufs=2)
                    nc.tensor.matmul(hp_ps, w1_r[:, ts(j, 128)], kT_r,
                                     start=True, stop=True)
                    nc.scalar.activation(out=hact_h[:, j, :], in_=hp_ps,
                                         func=AF.Gelu)
                    nc.scalar.activation(out=dgel[:, j, :], in_=hp_ps,
                                         func=AF.Derivative_Gelu)
                hh_ps = psA.tile([128, 512], F32, tag="proj", bufs=2)
                for j in range(4):
                    nc.tensor.matmul(hh_ps, w2_h[:, ts(j, 128)], hact_h[:, j, :],
                                     start=(j == 0), stop=(j == 3))
                hhsb = pA.tile([128, 512], F32, tag="hhsb")
                nc.vector.tensor_copy(out=hhsb, in_=hh_ps)
                sq2 = pA.tile([128, 512], F16, tag="sq2", bufs=2)
                nc.scalar.activation(out=sq2, in_=hh_ps, func=AF.Square)
                ms_ps = psA.tile([1, 512], F32, tag="mix", bufs=2)
                nc.tensor.matmul(ms_ps, ones_col_h, sq2, start=True, stop=True)
                rowt2 = pA.tile([1, 512], F32, tag="rows", bufs=16)
                nc.scalar.activation(out=rowt2, in_=ms_ps, func=AF.Sqrt,
                                     scale=1.0 / DH, bias=epsT[0:1, :])
                srs_f = pA.tile([1, 512], F32, tag="rows", bufs=16)
                nc.vector.reciprocal(out=srs_f, in_=rowt2)
                srs_h = pA.tile([1, 512], F16, tag="rows", bufs=16)
                nc.scalar.copy(out=srs_h, in_=srs_f)
                srsb_ps = psA.tile([128, 512], F32, tag="bc", bufs=2)
                nc.tensor.matmul(srsb_ps, ones_row_h, srs_h, start=True, stop=True)
                ysb = pA.tile([128, 512], F32, tag="ysb")
                nc.vector.tensor_mul(out=ysb, in0=hhsb, in1=srsb_ps)
                dp = pA.tile([128, 512], F32, tag="dp")
                nc.vector.scalar_tensor_tensor(out=dp, in0=ysb, scalar=gamma,
                                               in1=kvT, op0=AX.mult, op1=AX.add)
                nc.vector.tensor_mul(out=dp, in0=dp, in1=lrB)
                gp = pA.tile([128, 512], F32, tag="gp", bufs=2)
                nc.vector.tensor_mul(out=gp, in0=dp, in1=ysb)
                gG8 = pA.tile([128, 8], F32, tag="gG8", bufs=2)
                nc.vector.tensor_reduce(out=gG8,
                                        in_=gp.rearrange("p (c k) -> p c k", k=CHUNK),
                                        axis=X_AXIS, op=AX.add)
                gG8s = pA.tile([128, 8], F32, tag="gG8s", bufs=2)
                nc.vector.tensor_scalar_mul(out=gG8s, in0=gG8, scalar1=-2.0 / DH)
                dma(md_st[:, ds(a8, 8), 2:3].rearrange("p c x -> p (c x)"),
                    gG8s)
                dY = pA.tile([128, 512], F32, tag="dY")
                nc.vector.tensor_scalar_mul(out=dY, in0=dp, scalar1=gamma)
                dprod = pA.tile([128, 512], F16, tag="dprod", bufs=2)
                nc.vector.tensor_mul(out=dprod, in0=dY, in1=hhsb)
                dot_ps = psA.tile([1, 512], F32, tag="mix", bufs=2)
                nc.tensor.matmul(dot_ps, ones_col_h, dprod, start=True, stop=True)
                s3 = pA.tile([1, 512], F32, tag="rows", bufs=16)
                nc.vector.tensor_mul(out=s3, in0=srs_f, in1=srs_f)
                nc.vector.tensor_mul(out=s3, in0=s3, in1=srs_f)
                c_f = pA.tile([1, 512], F32, tag="rows", bufs=16)
                nc.vector.tensor_mul(out=c_f, in0=s3, in1=dot_ps)
                c_h = pA.tile([1, 512], F16, tag="rows", bufs=16)
                nc.scalar.activation(out=c_h, in_=c_f, func=AF.Copy, scale=1.0 / DH)
                cb_ps = psA.tile([128, 512], F32, tag="bc", bufs=2)
                nc.tensor.matmul(cb_ps, ones_row_h, c_h, start=True, stop=True)
                m1t = pA.tile([128, 512], F32, tag="m1t", bufs=2)
                nc.vector.tensor_mul(out=m1t, in0=dY, in1=srsb_ps)
                m2t = pA.tile([128, 512], F32, tag="m2t", bufs=2)
                nc.vector.tensor_mul(out=m2t, in0=hhsb, in1=cb_ps)
                dhh_h = pA.tile([128, 512], F16, tag="dhh_h")
                nc.vector.tensor_sub(out=dhh_h, in0=m1t, in1=m2t)

                # backward to dhpre (fp16)
                dhpre_h = pA.tile([128, 4, 512], F16, tag="dhpre_h")
                for j in range(4):
                    da_ps = psA.tile([128, 512], F32, tag="proj", bufs=2)
                    nc.tensor.matmul(da_ps, w2T_h[:, ts(j, 128)], dhh_h,
                                     start=True, stop=True)
                    nc.vector.tensor_mul(out=dhpre_h[:, j, :], in0=da_ps,
                                         in1=dgel[:, j, :])

                # token-major transposes (fp16) -> staging -> chunk-major DRAM
                st_kc = pA.tile([128, 4, 128], F16, tag="st_kc", bufs=1)
                st_dh = pA.tile([128, 4, 128], F16, tag="st_dh", bufs=1)
                st_dp = pA.tile([128, 4, 512], F16, tag="st_dp", bufs=1)
                st_ha = pA.tile([128, 4, 512], F16, tag="st_ha", bufs=1)
                for blk in range(4):
                    bsl = ts(blk, 128)
                    tp_ps = psA.tile([128, 4, 128], F16, tag="tp", bufs=2)
                    nc.tensor.transpose(tp_ps[:, 0, :], kT_h[:, bsl], ident_h)
                    nc.tensor.transpose(tp_ps[:, 1, :], dhh_h[:, bsl], ident_h)
                    nc.vector.tensor_copy(out=st_kc[:, blk, :], in_=tp_ps[:, 0, :])
                    nc.vector.tensor_copy(out=st_dh[:, blk, :], in_=tp_ps[:, 1, :])
                    for j in range(4):
                        t2_ps = psA.tile([128, 4, 128], F16, tag="tp", bufs=2)
                        nc.tensor.transpose(t2_ps[:, 0, :], dhpre_h[:, j, bsl],
                                            ident_h)
                        nc.tensor.transpose(t2_ps[:, 1, :], hact_h[:, j, bsl],
                                            ident_h)
                        nc.vector.tensor_copy(out=st_dp[:, blk, ts(j, 128)],
                                              in_=t2_ps[:, 0, :])
                        nc.vector.tensor_copy(out=st_ha[:, blk, ts(j, 128)],
                                              in_=t2_ps[:, 1, :])
                a4 = tt * 4
                for cm, stg in [(kc_st, st_kc), (dhh_st, st_dh),
                                (dhpre_st, st_dp), (hact_st, st_ha)]:
                    v = cm.rearrange("p (a two) x -> p a two x", two=2)
                    dma(v[:, ds(a4, 4), 0, :], stg[0:64, :, :])
                    dma(v[:, ds(a4, 4), 1, :], stg[64:128, :, :])

        # ============ CHUNK LOOP: retrieval + grads + NS5 + scans ============
        with tc.tile_pool(name="phL", bufs=1) as pL, \
             tc.tile_pool(name="psL", bufs=1, space="PSUM") as psL:
            with tc.For_i(0, NCH, 1) as c:
                kc_t = pL.tile([64, 128], F16, tag="kc_t", bufs=2)
                dma(kc_t, kc_st[:, ds(c, 1), :].rearrange("p one x -> p (one x)"))
                dhh_t = pL.tile([64, 128], F16, tag="dhh_t", bufs=2)
                dma(dhh_t, dhh_st[:, ds(c, 1), :].rearrange("p one x -> p (one x)"))
                dhpre_t = pL.tile([64, 512], F16, tag="dhpre_t", bufs=2)
                dma(dhpre_t,
                    dhpre_st[:, ds(c, 1), :].rearrange("p one x -> p (one x)"))
                hact_t = pL.tile([64, 512], F16, tag="hact_t", bufs=2)
                dma(hact_t,
                    hact_st[:, ds(c, 1), :].rearrange("p one x -> p (one x)"))
                q_t = pL.tile([128, CHUNK], F16, tag="q_t", bufs=2)
                dma(q_t, q_st[:, ds(c, 1), :].rearrange("p one x -> p (one x)"))
                gate_t = pL.tile([128, CHUNK], F16, tag="gate_t", bufs=2)
                dma(gate_t, g_st[:, ds(c, 1), :].rearrange("p one x -> p (one x)"))
                md_t = pL.tile([128, 4], F32, tag="md_t", bufs=2)
                dma(md_t, md_st[:, ds(c, 1), :].rearrange("p one x -> p (one x)"))

                # ---- retrieval with pre-update state ----
                hp_ps = psL.tile([128, 4, CHUNK], F32, tag="pr", bufs=1)
                for j in range(4):
                    nc.tensor.matmul(hp_ps[:, j, :], u1h[:, ts(j, 128)], q_t,
                                     start=True, stop=True)
                ha_c = pL.tile([128, 4, CHUNK], F16, tag="ha_c", bufs=2)
                nc.scalar.activation(out=ha_c, in_=hp_ps, func=AF.Gelu)
                hh_ps = psL.tile([128, CHUNK], F32, tag="pr", bufs=1)
                for j in range(4):
                    nc.tensor.matmul(hh_ps, u2h[:, ts(j, 128)], ha_c[:, j, :],
                                     start=(j == 0), stop=(j == 3))
                sqc = pL.tile([128, CHUNK], F16, tag="sqc", bufs=2)
                nc.scalar.activation(out=sqc, in_=hh_ps, func=AF.Square)
                hhc = pL.tile([128, CHUNK], F32, tag="hhc", bufs=2)
                nc.scalar.copy(out=hhc, in_=hh_ps)
                ms_ps = psL.tile([1, CHUNK], F32, tag="prow", bufs=1)
                nc.tensor.matmul(ms_ps, ones_col_h, sqc, start=True, stop=True)
                rr = pL.tile([1, CHUNK], F32, tag="rr", bufs=2)
                nc.scalar.activation(out=rr, in_=ms_ps, func=AF.Sqrt,
                                     scale=1.0 / DH, bias=epsT[0:1, :])
                rr2 = pL.tile([1, CHUNK], F32, tag="rr2", bufs=2)
                nc.vector.reciprocal(out=rr2, in_=rr)
                rrh = pL.tile([1, CHUNK], F16, tag="rrh", bufs=2)
                nc.scalar.copy(out=rrh, in_=rr2)
                sb_ps = psL.tile([128, CHUNK], F32, tag="pr", bufs=1)
                nc.tensor.matmul(sb_ps, ones_row_h, rrh, start=True, stop=True)
                yc = pL.tile([128, CHUNK], F32, tag="yc", bufs=2)
                nc.vector.tensor_mul(out=yc, in0=hhc, in1=sb_ps)
                prc = pL.tile([128, CHUNK], F32, tag="prc", bufs=2)
                nc.vector.scalar_tensor_tensor(out=prc, in0=yc, scalar=ugv,
                                               in1=q_t, op0=AX.mult, op1=AX.add)
                outc = pL.tile([128, CHUNK], F16, tag="outc", bufs=2)
                nc.vector.tensor_mul(out=outc, in0=prc, in1=gate_t)
                dma(d["out"].ap()[ds(c, 1)].rearrange("one p x -> (one p) x"),
                    outc)

                # ---- inner grads (g1 = dL/dw1, g2t = (dL/dw2)^T, both [dh,hid])
                g1_ps = psL.tile([128, 512], F32, tag="pg", bufs=2)
                nc.tensor.matmul(g1_ps, kc_t, dhpre_t, start=True, stop=True)
                g2_ps = psL.tile([128, 512], F32, tag="pg", bufs=2)
                nc.tensor.matmul(g2_ps, dhh_t, hact_t, start=True, stop=True)
                g1T_ps = psL.tile([128, 4, 128], F32, tag="pgT", bufs=2)
                for j in range(4):
                    nc.tensor.matmul(g1T_ps[:, j, :], dhpre_t[:, ts(j, 128)],
                                     kc_t, start=True, stop=True)
                g2T_ps = psL.tile([128, 4, 128], F32, tag="pgT", bufs=2)
                for j in range(4):
                    nc.tensor.matmul(g2T_ps[:, j, :], hact_t[:, ts(j, 128)],
                                     dhh_t, start=True, stop=True)
                g1sb = pL.tile([128, 512], F32R, tag="g1sb", bufs=2)
                nc.vector.tensor_copy(out=g1sb, in_=g1_ps)
                g2sb = pL.tile([128, 512], F32R, tag="g2sb", bufs=2)
                nc.vector.tensor_copy(out=g2sb, in_=g2_ps)
                R = pL.tile([128, 2], F32, tag="R", bufs=2)
                scr = pL.tile([128, 512], F16, tag="scr", bufs=2)
                nc.vector.scalar_tensor_tensor(
                    out=scr, in0=g1sb.bitcast(F32), scalar=1.0,
                    in1=g1sb.bitcast(F32), op0=AX.mult, op1=AX.mult,
                    accum_out=R[:, 0:1])
                scr2 = pL.tile([128, 512], F16, tag="scr", bufs=2)
                nc.vector.scalar_tensor_tensor(
                    out=scr2, in0=g2sb.bitcast(F32), scalar=1.0,
                    in1=g2sb.bitcast(F32), op0=AX.mult, op1=AX.mult,
                    accum_out=R[:, 1:2])
                Rh = pL.tile([128, 2], F16, tag="Rh", bufs=2)
                nc.vector.tensor_copy(out=Rh, in_=R)
                nrm_ps = psL.tile([1, 2], F32, tag="prow", bufs=1)
                nc.tensor.matmul(nrm_ps, ones_col_h, Rh, start=True, stop=True)
                nrm_sb = pL.tile([1, 2], F32, tag="nrm_sb", bufs=2)
                nc.vector.tensor_copy(out=nrm_sb, in_=nrm_ps)
                inv2 = pL.tile([1, 2], F32, tag="inv2", bufs=2)
                nc.vector.reciprocal_approx_fast(inv2, nrm_sb)
                ninv = pL.tile([1, 2], F32, tag="ninv", bufs=2)
                nc.scalar.activation(out=ninv, in_=inv2, func=AF.Sqrt)
                nc.scalar.activation(out=ninv, in_=ninv, func=AF.Copy, scale=-1.0)
                nb = pL.tile([128, 2], F32, tag="nb", bufs=2)
                nc.gpsimd.partition_broadcast(nb, ninv)

                # ---- NS5, fp32 (f32r matmuls), transpose-free ----
                tP = [None, None]
                tT = [None, None]
                tP[0] = pL.tile([128, 512], F32R, tag="tPa", bufs=2, name="tP0")
                nc.vector.tensor_scalar_mul(out=tP[0], in0=g1sb.bitcast(F32),
                                            scalar1=nb[:, 0:1])
                tT[0] = pL.tile([128, 4, 128], F32R, tag="tTa", bufs=2, name="tT0")
                nc.vector.tensor_scalar_mul(out=tT[0], in0=g1T_ps,
                                            scalar1=nb[:, 0:1])
                tP[1] = pL.tile([128, 512], F32R, tag="tPb", bufs=2, name="tP1")
                nc.vector.tensor_scalar_mul(out=tP[1], in0=g2sb.bitcast(F32),
                                            scalar1=nb[:, 1:2])
                tT[1] = pL.tile([128, 4, 128], F32R, tag="tTb", bufs=2, name="tT1")
                nc.vector.tensor_scalar_mul(out=tT[1], in0=g2T_ps,
                                            scalar1=nb[:, 1:2])

                for k in range(5):
                    last = k == 4
                    for i in range(2):
                        A_ps = psL.tile([128, 128], F32, tag="pA", bufs=2)
                        for j in range(4):
                            nc.tensor.matmul(A_ps, tT[i][:, j, :], tT[i][:, j, :],
                                             start=(j == 0), stop=(j == 3))
                        Ab = pL.tile([128, 128], F32R, tag="Ab", bufs=2)
                        nc.vector.tensor_scalar_mul(out=Ab, in0=A_ps, scalar1=NSB)
                        Au = pL.tile([128, 128], F32R, tag="Au", bufs=2)
                        nc.scalar.copy(out=Au, in_=A_ps)
                        A2_ps = psL.tile([128, 128], F32, tag="pA", bufs=2)
                        nc.tensor.matmul(A2_ps, Ab, Au, start=True, stop=False)
                        nc.tensor.matmul(A2_ps, identr, aIc, start=False, stop=True)
                        Bm = pL.tile([128, 128], F32R, tag="Bm", bufs=2)
                        nc.vector.scalar_tensor_tensor(
                            out=Bm, in0=A2_ps, scalar=NSC / NSB,
                            in1=Ab.bitcast(F32), op0=AX.mult, op1=AX.add)
                        if not (last and i == 1):
                            tp_ps = psL.tile([128, 512], F32, tag="pg", bufs=2)
                            nc.tensor.matmul(tp_ps, Bm, tP[i], start=True,
                                             stop=True)
                            tPn = pL.tile([128, 512], F32R,
                                          tag=("tPa" if i == 0 else "tPb"), bufs=2)
                            nc.scalar.copy(out=tPn, in_=tp_ps)
                        else:
                            tPn = tP[i]
                        if not (last and i == 0):
                            tt_ps = psL.tile([128, 4, 128], F32, tag="pgT", bufs=2)
                            for j in range(4):
                                nc.tensor.matmul(tt_ps[:, j, :],
                                                 tP[i][:, ts(j, 128)], Bm,
                                                 start=True, stop=True)
                            tTn = pL.tile([128, 4, 128], F32R,
                                          tag=("tTa" if i == 0 else "tTb"), bufs=2)
                            nc.scalar.copy(out=tTn, in_=tt_ps)
                        else:
                            tTn = tT[i]
                        tP[i] = tPn
                        tT[i] = tTn

                s1 = tP[0].bitcast(F32)
                s2 = tT[1].rearrange("p a b -> p (a b)").bitcast(F32)

                # ---- scans (momentum then weight-decay), fp32 ----
                nc.vector.scalar_tensor_tensor(out=m1s, in0=m1s,
                                               scalar=md_t[:, 0:1], in1=s1,
                                               op0=AX.mult, op1=AX.add)
                nc.vector.scalar_tensor_tensor(out=u1, in0=u1,
                                               scalar=md_t[:, 1:2], in1=m1s,
                                               op0=AX.mult, op1=AX.add)
                nc.scalar.copy(out=u1h, in_=u1)
                nc.vector.scalar_tensor_tensor(out=m2s, in0=m2s,
                                               scalar=md_t[:, 0:1], in1=s2,
                                               op0=AX.mult, op1=AX.add)
                nc.vector.scalar_tensor_tensor(out=u2, in0=u2,
                                               scalar=md_t[:, 1:2], in1=m2s,
                                               op0=AX.mult, op1=AX.add)
                nc.scalar.copy(out=u2h, in_=u2)
                nc.vector.scalar_tensor_tensor(out=mgv, in0=mgv,
                                               scalar=md_t[:, 0:1],
                                               in1=md_t[:, 2:3],
                                               op0=AX.mult, op1=AX.add)
                nc.vector.scalar_tensor_tensor(out=ugv, in0=ugv,
                                               scalar=md_t[:, 1:2], in1=mgv,
                                               op0=AX.mult, op1=AX.add)


# ------------------- host side -------------------

def _prep_core_inputs(inputs, b, h):
    f = np.float32
    sg = np.asarray(inputs["store_g"], f)[:, None]
    rg = np.asarray(inputs["retrieve_g"], f)[:, None]
    hs = slice(h * DH, (h + 1) * DH)

    def tile128(w):  # (512, X) -> rows grouped as (128, 4, X) -> (128, 4*X)
        w = np.asarray(w, f)
        return np.ascontiguousarray(
            w.reshape(4, 128, -1).transpose(1, 0, 2).reshape(128, -1))

    wk = tile128(sg * np.asarray(inputs["Wk"], f)[:, hs])
    wv = tile128(sg * np.asarray(inputs["Wv"], f)[:, hs])
    wq = tile128(rg * np.asarray(inputs["Wq"], f)[:, hs])
    wsm = tile128(np.stack([
        sg[:, 0] * np.asarray(inputs["W_lr"], f)[:, h],
        sg[:, 0] * np.asarray(inputs["Wm"], f)[:, h],
        sg[:, 0] * np.asarray(inputs["Wd"], f)[:, h],
        rg[:, 0] * np.asarray(inputs["Wgate"], f)[:, h]], axis=1))
    ident = np.eye(128, dtype=f)
    w1 = np.asarray(inputs["mw1"], f)[h]
    w2 = tile128(np.asarray(inputs["mw2"], f)[h])
    cw16 = np.concatenate([wk, wv, wq, wsm, ident, w1, w2],
                          axis=1).astype(np.float16)

    gamma = np.asarray(inputs["mgamma"], f)[h].reshape(128, 1)
    biasB = np.broadcast_to(
        np.array([inputs["b_lr"][h], 0.0, 0.0, 0.0], f), (128, 4))
    mdcol = np.zeros((128, 1), f)
    mdcol[0, 0] = inputs["bm"][h]
    mdcol[1, 0] = inputs["bd"][h]
    cw32 = np.ascontiguousarray(
        np.concatenate([gamma, biasB, mdcol], axis=1), f)

    seqq = np.asarray(inputs["seq"], f)[b, h * (N // 4):(h + 1) * (N // 4), :] \
        .T.astype(np.float16)
    return {"seqq": np.ascontiguousarray(seqq), "cw16": cw16, "cw32": cw32}


_CACHE = {}


def _get_module():
    if "nc" not in _CACHE:
        nc = bacc.Bacc("TRN2", target_bir_lowering=False, debug=False,
                       num_devices=8)
        build(nc)
        nc.compile()
        _CACHE["nc"] = nc
    return _CACHE["nc"]


def kernel(**inputs):
    from concourse.bass_utils import run_bass_kernel_spmd
    nc = _get_module()
    in_maps = [_prep_core_inputs(inputs, core // HEADS, core % HEADS)
               for core in range(8)]
    res = run_bass_kernel_spmd(nc, in_maps, core_ids=list(range(8)))
    _CACHE["last_res"] = res
    Wc = np.asarray(inputs["Wc"], np.float32)
    out = np.zeros((B, N, DIM), np.float32)
    for b in range(B):
        heads = []
        for h in range(HEADS):
            oc = res.results[b * HEADS + h]["out"]  # (NCH, 128, CHUNK) f16
            heads.append(
                oc.transpose(0, 2, 1).reshape(N, DH).astype(np.float32))
        out[b] = np.concatenate(heads, axis=1) @ Wc
    return out


if __name__ == "__main__":
    dd = np.load("/root/problem/ref_inputs.npz")
    inputs = {k: dd[k] for k in dd.files}
    out = kernel(**inputs)
    exp = np.load("/root/problem/ref_expected.npy")
    err = np.abs(out - exp).max() / np.abs(exp).max()
    rel = np.linalg.norm(out - exp) / np.linalg.norm(exp)
    print(f"absmax-rel: {err:.3e}  l2-rel: {rel:.3e}")
